# revision 13
# baseline (speedup 1.0000x reference)
"""Bass/Tile Trainium2 kernel for a 2-layer dense multi-head GAT over a batch
of B=8 independent subgraphs (2048 nodes each, equal contiguous segments).

Sharding: one subgraph per NeuronCore (8 cores), parameters replicated.

Algorithm (per core / subgraph, per attention layer):
  scores are rank-1:  e_ij = leaky_relu(s1_i + s2_j),  s1 = h@a1, s2 = h@a2.
  exp(leaky_relu(t)) is separable through the sign mask M_ij = [s1_i+s2_j>=0]:
      p_ij = M_ij e^{s1_i} e^{s2_j} + (1-M_ij) e^{a s1_i} e^{a s2_j}
  so softmax(e) @ h needs NO N^2 exp work:
      num_i = g_i (M @ u)_i + (vtot - (M @ v))_i          (e^{a s1} cancels in
      u_j = e^{s2_j} [h_j|1],  v_j = e^{a s2_j} [h_j|1],   the Z ratio; g =
      out_i = num_i[:64] / num_i[64]                       e^{(1-a) s1})
  The N^2 work is the 0/1 mask build (exact in bf16; DVE is_ge in its 2x
  bf16 mode, with the tail jc columns on GPSIMD) plus mask matmuls with a
  single bf16 [u|-v] stream.  Each head's projections/exponentials are
  emitted inside the previous head's attention quarters so every engine
  stays busy across head boundaries; the layer-2 transposes/projections are
  likewise pipelined into head 3's quarters, and the log_softmax tail is
  per-quarter so the output DMA overlaps the last attention.
"""

from contextlib import ExitStack

import numpy as np

import concourse.bass as bass
import concourse.tile as tile
from concourse import bacc, mybir
from concourse.masks import make_identity

FP = mybir.dt.float32
BF = mybir.dt.bfloat16
AF = mybir.ActivationFunctionType
OP = mybir.AluOpType

B = 8
N = 2048
D = 64
H = 4
ALPHA = 0.2
P = 128
NCH = N // P  # 16 chunks of 128 nodes
DEXT = D + 1  # h plus ones column

# mask ownership: GPSIMD owns the tail jc columns of each half (it needs the
# longest runway; its tiles are emitted during the PREVIOUS head's quarters)
GP_JC = {12, 13, 14, 15}
GP_JC_L2 = {14, 15}  # ACT carries escr in layer 2; GPSIMD picks up onorm


def _emit_prep_exps(nc, prep, st, cgs):
    """es2/nes2 exponentials for s12 column groups cgs (4 cols each)."""
    for cg in cgs:
        gs = slice(cg * 4, (cg + 1) * 4)
        nc.scalar.activation(st["es2"][:, gs], st["s12"][:, gs, 1], AF.Exp)
        nc.scalar.activation(st["nes2"][:, gs], st["s12"][:, gs, 1], AF.Exp,
                             scale=ALPHA)


def _emit_uv(nc, st, chunks):
    """u | -v bf16 stream for node chunks (the -1 rides the 2nd ALU op)."""
    uv, hext = st["uv"], st["hext"]
    for c in chunks:
        nc.vector.tensor_scalar(uv[:, c, 0:DEXT], hext[:, c, :],
                                st["es2"][:, c:c + 1], None, OP.mult)
        nc.vector.tensor_scalar(uv[:, c, DEXT:], hext[:, c, :],
                                st["nes2"][:, c:c + 1], -1.0, OP.mult, OP.mult)


def _emit_vrows(nc, prep, psaux, scratch, st, tag):
    """vtot = sum_j v_j via PE; bf16 hi+res seed rows [1, 130]."""
    vt_ps = psaux.tile([1, DEXT], FP, tag="aux", name=f"vt_{tag}")
    for c in range(NCH):
        nc.tensor.matmul(vt_ps, scratch["ones_col_bf"], st["uv"][:, c, DEXT:],
                         start=(c == 0), stop=(c == NCH - 1))
    vrow = prep.tile([1, 2 * DEXT], BF, tag="vrow", name=f"vrow_{tag}")
    nc.vector.memset(vrow[:, 0:DEXT], 0.0)
    nc.vector.tensor_scalar(vrow[:, DEXT:], vt_ps, -1.0, None, OP.mult)
    vres = prep.tile([1, 2 * DEXT], BF, tag="vres", name=f"vres_{tag}")
    nc.vector.memset(vres[:, 0:DEXT], 0.0)
    nc.vector.scalar_tensor_tensor(vres[:, DEXT:], vt_ps, -1.0,
                                   vrow[:, DEXT:], OP.mult, OP.subtract)
    st["vrow"], st["vres"] = vrow, vres


def _emit_gp_masks(nc, mask_gp, st, gp_jc, tag):
    """GPSIMD-owned [P,1024] mask units, emitted as early as possible."""
    tiles = st.setdefault("gp_tiles", {})
    for half in range(2):
        for jc in sorted(gp_jc):
            mt = mask_gp.tile([P, 1024], BF, tag="mtg",
                              name=f"mtg{tag}_{half}_{jc}")
            nc.gpsimd.tensor_scalar(
                mt, st["s1b"][:, half * 1024:(half + 1) * 1024],
                st["s12"][:, jc, 1:2], 0.0, OP.add, OP.is_ge)
            tiles[(half, jc)] = mt


def _attention(nc, pools, scratch, st, out_cb, gp_jc, onorm_gp, tag,
               hook=None):
    """Masked-matmul attention core; st holds the layer's prepped tensors."""
    const, prep, mask_dve, mask_gp, wide, small, psA, psaux = pools
    s12, uv = st["s12"], st["uv"]
    g = st["g"]
    ones_row_bf = scratch["ones_row_bf"]

    nsum_w = wide.tile([P, NCH, DEXT], FP, tag="nsum", name=f"nsum_{tag}")
    onorm = wide.tile([P, NCH, D], FP, tag="onorm", name=f"onorm_{tag}")
    LOOKAHEAD = 7
    steps = [(q, jc) for q in range(4) for jc in range(NCH)]
    tiles = dict(st.get("gp_tiles", ()))  # (half, jc) -> [P,1024] tile

    def emit_step(step_idx):
        if step_idx >= len(steps):
            return
        q, jc = steps[step_idx]
        half = q // 2
        if (half, jc) in tiles:
            return
        mt = mask_dve.tile([P, 1024], BF, tag="mtd",
                           name=f"mtd{tag}_{half}_{jc}")
        nc.vector.tensor_scalar(mt, st["s1b"][:, half * 1024:(half + 1) * 1024],
                                s12[:, jc, 1:2], 0.0, OP.add, OP.is_ge)
        tiles[(half, jc)] = mt

    for i in range(LOOKAHEAD):
        emit_step(i)

    for q in range(4):  # quarters of the i (destination-node) axis
        A = [psA.tile([P, 2 * DEXT], FP, tag="A", name=f"A{tag}_{q}_{il}")
             for il in range(4)]
        half, off = q // 2, (q % 2) * 512
        for jc in range(NCH):
            mt = tiles[(half, jc)]
            if q == 2 * half + 1 and (half, jc) not in st.get("gp_tiles", ()):
                del tiles[(half, jc)]  # last use; free the ring slot
            emit_step(q * NCH + jc + LOOKAHEAD)
            for il in range(4):
                sl = mt[:, off + il * P:off + (il + 1) * P]
                nc.tensor.matmul(A[il], sl, uv[:, jc, :],
                                 start=(jc == 0), stop=False)
        # seed vtot last (PSUM accumulation is order-insensitive): two K=1
        # matmuls (hi + res rows) -- no cross-partition DMA hop needed
        for il in range(4):
            nc.tensor.matmul(A[il], ones_row_bf[0:1, :], st["vrow"],
                             start=False, stop=False)
            nc.tensor.matmul(A[il], ones_row_bf[0:1, :], st["vres"],
                             start=False, stop=True)
        qs = slice(q * 4, (q + 1) * 4)
        for il in range(4):
            ic = q * 4 + il
            # nsum = g * (M@u) + (vtot - M@v): ACT evacuates the u-half with
            # the g-scale fused (Copy+scale), DVE adds the PSUM w-half.
            nU = small.tile([P, DEXT], FP, tag="nU", name=f"nU{tag}_{ic}")
            nc.scalar.activation(nU, A[il][:, 0:DEXT], AF.Copy,
                                 scale=g[:, ic:ic + 1])
            nc.vector.tensor_tensor(nsum_w[:, ic, :], nU, A[il][:, DEXT:],
                                    OP.add)
        rz = small.tile([P, 4], FP, tag="rz", name=f"rz{tag}_{q}")
        nc.vector.reciprocal(rz, nsum_w[:, qs, D])
        for k in range(4):
            ic = q * 4 + k
            if onorm_gp:
                nc.gpsimd.tensor_scalar(onorm[:, ic, :], nsum_w[:, ic, 0:D],
                                        rz[:, k:k + 1], None, OP.mult)
            else:
                nc.scalar.activation(onorm[:, ic, :], nsum_w[:, ic, 0:D],
                                     AF.Copy, scale=rz[:, k:k + 1])
        out_cb(onorm, q)
        if hook is not None:
            hook(q)


def _elu_q(nc, wide, onorm, q, dst_writer, tag):
    """elu over quarter q of onorm [P, NCH, D]; writes via dst_writer(src)."""
    src = onorm[:, q * 4:(q + 1) * 4, :]
    m = wide.tile([P, 4, D], FP, tag="elu_m", name=f"elu_m{tag}{q}")
    nc.gpsimd.tensor_scalar(m, src, 0.0, None, OP.min)
    e = wide.tile([P, 4, D], FP, tag="elu_e", name=f"elu_e{tag}{q}")
    nc.scalar.activation(e, m, AF.Exp)
    r = wide.tile([P, 4, D], FP, tag="elu_r", name=f"elu_r{tag}{q}")
    nc.gpsimd.tensor_scalar(r, src, 0.0, -1.0, OP.max, OP.add)
    dst_writer(r, e)


def build_kernel():
    nc = bacc.Bacc("TRN2", target_bir_lowering=False, debug=False,
                   num_devices=B)

    x = nc.dram_tensor("x", [N, D], FP, kind="ExternalInput")
    W_heads = nc.dram_tensor("W_heads", [H, D, D], FP, kind="ExternalInput")
    a_heads = nc.dram_tensor("a_heads", [H, 2 * D], FP, kind="ExternalInput")
    W_out = nc.dram_tensor("W_out", [H * D, D], FP, kind="ExternalInput")
    a_out = nc.dram_tensor("a_out", [2 * D], FP, kind="ExternalInput")
    out = nc.dram_tensor("out", [N, D], FP, kind="ExternalOutput")

    with tile.TileContext(nc) as tc, ExitStack() as ctx:
        const = ctx.enter_context(tc.tile_pool(name="const", bufs=1))
        prep = ctx.enter_context(tc.tile_pool(name="prep", bufs=3))
        mask_dve = ctx.enter_context(tc.tile_pool(name="mask_dve", bufs=19))
        mask_gp = ctx.enter_context(tc.tile_pool(name="mask_gp", bufs=12))
        wide = ctx.enter_context(tc.tile_pool(name="wide", bufs=2))
        small = ctx.enter_context(tc.tile_pool(name="small", bufs=6))
        psA = ctx.enter_context(tc.tile_pool(name="psA", bufs=6, space="PSUM"))
        psaux = ctx.enter_context(tc.tile_pool(name="psaux", bufs=2,
                                               space="PSUM"))
        pools = (const, prep, mask_dve, mask_gp, wide, small, psA, psaux)

        ident = const.tile([P, P], FP)
        make_identity(nc, ident)
        ones128 = const.tile([P, P], FP)
        nc.vector.memset(ones128, 1.0)
        ones_col_bf = const.tile([P, 1], BF)
        nc.vector.memset(ones_col_bf, 1.0)
        ones_row_bf = const.tile([1, P], BF)
        nc.vector.memset(ones_row_bf, 1.0)
        scratch = {"ones128": ones128, "ones_col_bf": ones_col_bf,
                   "ones_row_bf": ones_row_bf}

        # ---- load inputs (x in 4 pieces so transposes start early) ----
        x_sb = const.tile([P, NCH, D], FP)
        x_r = x.rearrange("(c p) d -> p c d", p=P)
        for r4 in range(4):
            nc.sync.dma_start(out=x_sb[:, r4 * 4:(r4 + 1) * 4, :],
                              in_=x_r[:, r4 * 4:(r4 + 1) * 4, :])
        Wh = const.tile([64, H, D], FP)
        nc.sync.dma_start(out=Wh, in_=W_heads.rearrange("h k d -> k h d"))
        WhT = const.tile([64, H, D], FP)
        nc.sync.dma_start(out=WhT, in_=W_heads.rearrange("h k d -> d h k"))
        a_sb = const.tile([64, H, 2], FP)
        nc.sync.dma_start(out=a_sb, in_=a_heads.rearrange("h (t k) -> k h t", t=2))
        Wo = const.tile([P, 2, D], FP)
        nc.sync.dma_start(out=Wo, in_=W_out.rearrange("(c k) d -> k c d", k=P))
        WoT = const.tile([64, 2, P], FP)
        nc.sync.dma_start(out=WoT, in_=W_out.rearrange("(c k) d -> d c k", k=P))
        ao = const.tile([64, 2], FP)
        nc.sync.dma_start(out=ao, in_=a_out.rearrange("(t k) -> k t", t=2))

        # bf16 shadows of the moving matmul operands (4x cheaper PE rows)
        Wh_bf = const.tile([64, H, D], BF)
        nc.vector.tensor_copy(Wh_bf, Wh)
        Wo_bf = const.tile([P, 2, D], BF)
        nc.vector.tensor_copy(Wo_bf, Wo)

        # all heads' wa = W_h @ [a1|a2] upfront (re-association: s = x @ wa);
        # only needs the parameter DMAs, so it fills the startup bubble
        wa_all = const.tile([64, H, 2], FP)
        for h in range(H):
            wap = psaux.tile([64, 2], FP, tag="aux", name=f"wap{h}")
            nc.tensor.matmul(wap, WhT[:, h, :], a_sb[:, h, :], start=True,
                             stop=True)
            nc.scalar.copy(wa_all[:, h, :], wap)

        xT = const.tile([64, N], FP)
        xT_bf = const.tile([64, N], BF)

        def new_state(tag):
            st = {}
            st["s12"] = prep.tile([P, NCH, 2], FP, tag="s12",
                                  name=f"s12_{tag}")
            st["s1b"] = prep.tile([P, N], BF, tag="s1b", name=f"s1b_{tag}")
            st["hext"] = prep.tile([P, NCH, DEXT], BF, tag="hext",
                                   name=f"hext_{tag}")
            nc.vector.memset(st["hext"][:, :, D], 1.0)
            st["uv"] = prep.tile([P, NCH, 2 * DEXT], BF, tag="uv",
                                 name=f"uv_{tag}")
            st["es2"] = prep.tile([P, NCH], FP, tag="es2", name=f"es2_{tag}")
            st["nes2"] = prep.tile([P, NCH], FP, tag="nes2",
                                   name=f"nes2_{tag}")
            st["g"] = prep.tile([P, NCH], FP, tag="g", name=f"g_{tag}")
            return st

        def l1_partA(st, h, cgs=range(4)):
            # s12 columns (batched copies, 4 chunks per PSUM tile)
            wa = wa_all[:, h, :]
            for cg in cgs:
                sp = psaux.tile([P, 8], FP, tag="aux", name=f"sp{h}_{cg}")
                for k in range(4):
                    c = cg * 4 + k
                    nc.tensor.matmul(sp[:, 2 * k:2 * k + 2],
                                     xT[:, c * P:(c + 1) * P], wa,
                                     start=True, stop=True)
                nc.scalar.copy(st["s12"][:, cg * 4:(cg + 1) * 4, :], sp)

        def l1_partB(st, h, rs=range(4), halves=range(2)):
            # s1b (bf16 mask input): s1 row replicated via (ones*wa1) x xT
            wa = wa_all[:, h, :]
            wa1b = prep.tile([64, P], BF, tag="wa1b", name=f"wa1b_{h}")
            nc.vector.tensor_scalar(wa1b, ones128[0:64, :], wa[:, 0:1], None,
                                    OP.mult)
            for r in rs:
                ps = psaux.tile([P, 512], FP, tag="aux", name=f"s1p{h}_{r}")
                nc.tensor.matmul(ps, wa1b, xT_bf[:, r * 512:(r + 1) * 512],
                                 start=True, stop=True)
                nc.scalar.copy(st["s1b"][:, r * 512:(r + 1) * 512], ps)
            # h natural (+ones col), bf16, batched 8 chunks per PSUM bank
            for half in halves:
                hp = psaux.tile([P, 8, D], FP, tag="aux", name=f"hp{h}_{half}")
                for k in range(8):
                    c = half * 8 + k
                    nc.tensor.matmul(hp[:, k, :], xT_bf[:, c * P:(c + 1) * P],
                                     Wh_bf[:, h, :], start=(k == 0),
                                     stop=(k == 7))
                nc.scalar.copy(st["hext"][:, half * 8:(half + 1) * 8, 0:D], hp)

        def l1_partC(st, h):
            _emit_prep_exps(nc, prep, st, range(4))
            nc.scalar.activation(st["g"], st["s12"][:, :, 0], AF.Exp,
                                 scale=1.0 - ALPHA)
            _emit_uv(nc, st, range(NCH))
            _emit_vrows(nc, prep, psaux, scratch, st, f"h{h}")
            _emit_gp_masks(nc, mask_gp, st, GP_JC, f"h{h}")

        # ---- startup: interleave x transposes with head-0 prep ----
        st0 = new_state("h0")
        for r4 in range(4):
            for k in range(4):
                c = r4 * 4 + k
                tp = psaux.tile([64, P], FP, tag="aux", name=f"tp{c}")
                nc.tensor.transpose(tp, x_sb[:, c, :], ident)
                if c % 2 == 0:
                    nc.vector.tensor_copy(xT[:, c * P:(c + 1) * P], tp)
                else:
                    nc.scalar.copy(xT[:, c * P:(c + 1) * P], tp)
            nc.vector.tensor_copy(xT_bf[:, r4 * 512:(r4 + 1) * 512],
                                  xT[:, r4 * 512:(r4 + 1) * 512])
            l1_partA(st0, 0, cgs=[r4])
            l1_partB(st0, 0, rs=[r4], halves=[])
        l1_partB(st0, 0, rs=[], halves=range(2))
        l1_partC(st0, 0)

        # ---- layer 1: four heads -> xc01/xc23 ----
        xc01 = const.tile([P, NCH, 2, D], FP)
        xc23 = const.tile([P, NCH, 2, D], FP)
        xcT_bf = const.tile([P, 2, N], BF)

        # layer-2 state (filled by head-3 hooks)
        st2 = None
        states = [st0]

        def l2_boundary(hq):
            """Pipelined into head 3's quarters: transpose the finished xc
            quarter, then the layer-2 projections that depend on it."""
            st = st2
            for c in range(hq * 4, hq * 4 + 4):
                for kc, xc in ((0, xc01), (1, xc23)):
                    tp = psaux.tile([P, P], FP, tag="aux", name=f"tc{c}_{kc}")
                    nc.tensor.transpose(tp, xc[:, c, :, :], ident)
                    if (c + kc) % 2 == 0:
                        nc.vector.tensor_copy(
                            xcT_bf[:, kc, c * P:(c + 1) * P], tp)
                    else:
                        nc.scalar.copy(xcT_bf[:, kc, c * P:(c + 1) * P], tp)
            if hq == 0:
                # wa2 = W_out @ [a1|a2] (bf16 shadow for the bf16 stationary)
                wa2 = prep.tile([P, 2, 2], FP, tag="wa2")
                for kc in range(2):
                    wap = psaux.tile([P, 2], FP, tag="aux", name=f"wap2_{kc}")
                    nc.tensor.matmul(wap, WoT[:, kc, :], ao, start=True,
                                     stop=True)
                    nc.scalar.copy(wa2[:, kc, :], wap)
                wa2b = prep.tile([P, 2, 2], BF, tag="wa2b")
                nc.vector.tensor_copy(wa2b, wa2)
                st["wa2b"] = wa2b
                wa1b2 = prep.tile([P, 2, P], BF, tag="wa1b2")
                for kc in range(2):
                    nc.vector.tensor_scalar(wa1b2[:, kc, :], ones128,
                                            wa2[:, kc, 0:1], None, OP.mult)
                st["wa1b2"] = wa1b2
            # s12_2 for this chunk group
            cg = hq
            sp = psaux.tile([P, 8], FP, tag="aux", name=f"sp2_{cg}")
            for k in range(4):
                c = cg * 4 + k
                for kc in range(2):
                    nc.tensor.matmul(sp[:, 2 * k:2 * k + 2],
                                     xcT_bf[:, kc, c * P:(c + 1) * P],
                                     st["wa2b"][:, kc, :],
                                     start=(kc == 0), stop=(kc == 1))
            nc.scalar.copy(st["s12"][:, cg * 4:(cg + 1) * 4, :], sp)
            _emit_prep_exps(nc, prep, st, [cg])
            # s1b_2 piece hq (needs xcT chunks 4hq..4hq+3)
            r = hq
            ps = psaux.tile([P, 512], FP, tag="aux", name=f"s1p2_{r}")
            for kc in range(2):
                nc.tensor.matmul(ps, st["wa1b2"][:, kc, :],
                                 xcT_bf[:, kc, r * 512:(r + 1) * 512],
                                 start=(kc == 0), stop=(kc == 1))
            nc.scalar.copy(st["s1b"][:, r * 512:(r + 1) * 512], ps)
            # h2ext halves once their 8 chunks are transposed
            if hq in (1, 3):
                half = hq // 2
                hp = psaux.tile([P, 8, D], FP, tag="aux", name=f"hp2_{half}")
                for k in range(8):
                    c = half * 8 + k
                    for kc in range(2):
                        nc.tensor.matmul(hp[:, k, :],
                                         xcT_bf[:, kc, c * P:(c + 1) * P],
                                         Wo_bf[:, kc, :],
                                         start=(k == 0 and kc == 0),
                                         stop=(k == 7 and kc == 1))
                nc.scalar.copy(st["hext"][:, half * 8:(half + 1) * 8, 0:D],
                               hp)
                _emit_uv(nc, st, range(half * 8, half * 8 + 8))
            if hq == 3:
                nc.scalar.activation(st["g"], st["s12"][:, :, 0], AF.Exp,
                                     scale=1.0 - ALPHA)
                _emit_vrows(nc, prep, psaux, scratch, st, "l2")
                _emit_gp_masks(nc, mask_gp, st, GP_JC_L2, "l2")

        for h in range(H):
            st = states[h]

            def l1_out(onorm, q, h=h):
                xc = xc01 if h < 2 else xc23

                def write(r, e):
                    nc.vector.tensor_tensor(
                        xc[:, q * 4:(q + 1) * 4, h % 2, :], r, e, OP.add)

                _elu_q(nc, wide, onorm, q, write, f"h{h}")

            if h < H - 1:
                nst = new_state(f"h{h + 1}")
                states.append(nst)

                def hook(q, h=h, nst=nst):
                    if q == 1:
                        l1_partA(nst, h + 1)
                    elif q == 2:
                        l1_partB(nst, h + 1)
                    elif q == 3:
                        l1_partC(nst, h + 1)
            else:
                st2 = new_state("l2")

                def hook(q):
                    l2_boundary(q)

            _attention(nc, pools, scratch, st, l1_out, GP_JC, False,
                       f"h{h}", hook)

        # ---- layer 2 attention + elu + per-quarter log_softmax -> out ----
        out_w = const.tile([P, NCH, D], FP)
        out_r = out.rearrange("(c p) d -> p c d", p=P)
        o2_all = const.tile([P, NCH, D], FP)
        esum_all = const.tile([P, NCH], FP)
        lse = const.tile([P, NCH], FP)

        def l2_out(onorm, q):
            # elu + raw exp-sum (elu output is <= ~20, so exp is fp32-safe
            # without max subtraction), then the per-quarter softmax tail
            qs = slice(q * 4, (q + 1) * 4)
            o2 = o2_all[:, qs, :]

            def write(r, e):
                nc.vector.tensor_tensor(o2, r, e, OP.add)

            _elu_q(nc, wide, onorm, q, write, "l2")
            escr = wide.tile([P, 4, D], FP, tag="escr", name=f"escr{q}")
            for k in range(4):
                ic = q * 4 + k
                nc.scalar.activation(escr[:, k, :], o2[:, k, :], AF.Exp,
                                     accum_out=esum_all[:, ic:ic + 1])
            nc.scalar.activation(lse[:, qs], esum_all[:, qs], AF.Ln)
            lse_b = bass.AP(tensor=lse.tensor, offset=lse.offset + q * 4,
                            ap=[lse.ap[0], [lse.ap[1][0], 4], [0, D]])
            nc.vector.tensor_tensor(out_w[:, qs, :], o2, lse_b, OP.subtract)
            nc.sync.dma_start(out=out_r[:, qs, :], in_=out_w[:, qs, :])

        _attention(nc, pools, scratch, st2, l2_out, GP_JC_L2, True, "l2")

    nc.compile()
    return nc


_NC_CACHE = {}


def _make_runner(nc):
    """Build a cached sharded executable (run_bass_kernel_spmd re-traces
    jax.jit on every call; this jits once and reuses)."""
    import jax
    from jax.sharding import Mesh, PartitionSpec
    try:
        from jax.experimental.shard_map import shard_map
    except ImportError:
        from jax.shard_map import shard_map
    import concourse.mybir as mb
    from concourse import bass2jax

    bass2jax.install_neuronx_cc_hook()

    part_name = nc.partition_id_tensor.name if nc.partition_id_tensor else None
    in_names, out_names, out_avals = [], [], []
    for alloc in nc.m.functions[0].allocations:
        if not isinstance(alloc, mb.MemoryLocationSet):
            continue
        name = alloc.memorylocations[0].name
        if alloc.kind == "ExternalInput":
            if name != part_name:
                in_names.append(name)
        elif alloc.kind == "ExternalOutput":
            out_names.append(name)
            out_avals.append(jax.core.ShapedArray(
                tuple(alloc.tensor_shape), mb.dt.np(alloc.dtype)))
    n_params = len(in_names)
    all_names = in_names + out_names
    if part_name is not None:
        all_names = all_names + [part_name]

    def _body(*args):
        operands = list(args)
        if part_name is not None:
            operands.append(bass2jax.partition_id_tensor())
        return tuple(bass2jax._bass_exec_p.bind(
            *operands, out_avals=tuple(out_avals), in_names=tuple(all_names),
            out_names=tuple(out_names), lowering_input_output_aliases=(),
            sim_require_finite=True, sim_require_nnan=True, nc=nc))

    devices = jax.devices()[:B]
    mesh = Mesh(np.asarray(devices), ("core",))
    n_outs = len(out_names)
    sharded = jax.jit(
        shard_map(_body, mesh=mesh,
                  in_specs=(PartitionSpec("core"),) * (n_params + n_outs),
                  out_specs=(PartitionSpec("core"),) * n_outs,
                  check_rep=False),
        donate_argnums=tuple(range(n_params, n_params + n_outs)),
        keep_unused=True)

    def run(in_maps):
        concat_in = [
            np.concatenate([np.asarray(in_maps[c][nm])[None] for c in range(B)],
                           axis=0).reshape(B * in_maps[0][nm].shape[0],
                                           *in_maps[0][nm].shape[1:])
            for nm in in_names
        ]
        concat_zeros = [
            np.zeros((B * av.shape[0], *av.shape[1:]), av.dtype)
            for av in out_avals
        ]
        out_arrs = sharded(*concat_in, *concat_zeros)
        return [
            {nm: np.asarray(out_arrs[i]).reshape(B, *out_avals[i].shape)[c]
             for i, nm in enumerate(out_names)}
            for c in range(B)
        ]

    return run


def kernel(**inputs):
    h_states = np.ascontiguousarray(np.asarray(inputs["h_states"], dtype=np.float32))
    W_heads = np.ascontiguousarray(np.asarray(inputs["W_heads"], dtype=np.float32))
    a_heads = np.ascontiguousarray(np.asarray(inputs["a_heads"], dtype=np.float32))
    W_out = np.ascontiguousarray(np.asarray(inputs["W_out"], dtype=np.float32))
    a_out = np.ascontiguousarray(np.asarray(inputs["a_out"], dtype=np.float32))

    if "nc" not in _NC_CACHE:
        _NC_CACHE["nc"] = build_kernel()
        _NC_CACHE["run"] = _make_runner(_NC_CACHE["nc"])

    xs = h_states.reshape(B, N, D)
    in_maps = [
        {"x": xs[c], "W_heads": W_heads, "a_heads": a_heads,
         "W_out": W_out, "a_out": a_out}
        for c in range(B)
    ]
    results = _NC_CACHE["run"](in_maps)
    return np.concatenate([results[c]["out"] for c in range(B)], axis=0)


if __name__ == "__main__":
    # smoke test (self-contained: random inputs, shape/dtype check only)
    rng = np.random.default_rng(0)
    inputs = {
        "h_states": rng.standard_normal((B * N, D)).astype(np.float32),
        "W_heads": rng.standard_normal((H, D, D)).astype(np.float32) * 0.18,
        "a_heads": rng.standard_normal((H, 2 * D)).astype(np.float32) * 0.18,
        "W_out": rng.standard_normal((H * D, D)).astype(np.float32) * 0.09,
        "a_out": rng.standard_normal((2 * D,)).astype(np.float32) * 0.18,
        "seq_start_end": (np.arange(B, dtype=np.int32)[:, None] * N
                          + np.array([0, N], dtype=np.int32)[None, :]),
    }
    got = kernel(**inputs)
    print("kernel output", got.shape, got.dtype)


# revision 24
# speedup vs baseline: 1.0356x; 1.0356x over previous
"""Bass/Tile Trainium2 kernel for a 2-layer dense multi-head GAT over a batch
of B=8 independent subgraphs (2048 nodes each, equal contiguous segments).

Sharding: one subgraph per NeuronCore (8 cores), parameters replicated.

Algorithm (per core / subgraph, per attention layer):
  scores are rank-1:  e_ij = leaky_relu(s1_i + s2_j),  s1 = h@a1, s2 = h@a2.
  exp(leaky_relu(t)) is separable through the sign mask M_ij = [s1_i+s2_j>=0]:
      p_ij = M_ij e^{s1_i} e^{s2_j} + (1-M_ij) e^{a s1_i} e^{a s2_j}
  so softmax(e) @ h needs NO N^2 exp work:
      num_i = g_i (M @ u)_i + (vtot - (M @ v))_i          (e^{a s1} cancels in
      u_j = e^{s2_j} [h_j|1],  v_j = e^{a s2_j} [h_j|1],   the Z ratio; g =
      out_i = num_i[:64] / num_i[64]                       e^{(1-a) s1})
  The N^2 work is the 0/1 mask build (exact in bf16; DVE is_ge in its 2x
  bf16 mode, with the tail jc columns on GPSIMD) plus mask matmuls with a
  single bf16 [u|-v] stream.  Each head's projections/exponentials are
  emitted inside the previous head's attention quarters so every engine
  stays busy across head boundaries; the layer-2 transposes/projections are
  likewise pipelined into head 3's quarters, and the log_softmax tail is
  per-quarter so the output DMA overlaps the last attention.
"""

from contextlib import ExitStack

import numpy as np

import concourse.bass as bass
import concourse.tile as tile
from concourse import bacc, mybir
from concourse.masks import make_identity

FP = mybir.dt.float32
BF = mybir.dt.bfloat16
AF = mybir.ActivationFunctionType
OP = mybir.AluOpType

B = 8
N = 2048
D = 64
H = 4
ALPHA = 0.2
P = 128
NCH = N // P  # 16 chunks of 128 nodes
DEXT = D + 1  # h plus ones column

# mask ownership: GPSIMD owns the tail jc columns of each half (it needs the
# longest runway; its tiles are emitted during the PREVIOUS head's quarters).
# Head 0 and layer 2 have no / little runway for half 0, so GPSIMD only gets
# half-1 units there.
GP_MAIN = {(0, 12), (0, 13), (0, 14), (0, 15),
           (1, 12), (1, 13), (1, 14), (1, 15)}
GP_SHORT = {(1, 12), (1, 13), (1, 14), (1, 15)}


def _emit_prep_exps(nc, prep, st, cgs):
    """es2/nes2 exponentials for s12 column groups cgs (4 cols each)."""
    for cg in cgs:
        gs = slice(cg * 4, (cg + 1) * 4)
        nc.scalar.activation(st["es2"][:, gs], st["s12"][:, gs, 1], AF.Exp)
        nc.scalar.activation(st["nes2"][:, gs], st["s12"][:, gs, 1], AF.Exp,
                             scale=ALPHA)


def _emit_uv(nc, st, chunks):
    """u | -v bf16 stream for node chunks (the -1 rides the 2nd ALU op)."""
    uv, hext = st["uv"], st["hext"]
    for c in chunks:
        nc.vector.tensor_scalar(uv[:, c, 0:DEXT], hext[:, c, :],
                                st["es2"][:, c:c + 1], None, OP.mult)
        nc.vector.tensor_scalar(uv[:, c, DEXT:], hext[:, c, :],
                                st["nes2"][:, c:c + 1], -1.0, OP.mult, OP.mult)


def _emit_vrows(nc, prep, psaux, scratch, st, tag):
    """vtot = sum_j v_j via PE; bf16 hi+res seed rows [1, 130]."""
    vt_ps = psaux.tile([1, DEXT], FP, tag="aux", name=f"vt_{tag}")
    for c in range(NCH):
        nc.tensor.matmul(vt_ps, scratch["ones_col_bf"], st["uv"][:, c, DEXT:],
                         start=(c == 0), stop=(c == NCH - 1))
    vrow = prep.tile([1, 2 * DEXT], BF, tag="vrow", name=f"vrow_{tag}")
    nc.vector.memset(vrow[:, 0:DEXT], 0.0)
    nc.vector.tensor_scalar(vrow[:, DEXT:], vt_ps, -1.0, None, OP.mult)
    vres = prep.tile([1, 2 * DEXT], BF, tag="vres", name=f"vres_{tag}")
    nc.vector.memset(vres[:, 0:DEXT], 0.0)
    nc.vector.scalar_tensor_tensor(vres[:, DEXT:], vt_ps, -1.0,
                                   vrow[:, DEXT:], OP.mult, OP.subtract)
    st["vrow"], st["vres"] = vrow, vres


def _emit_gp_masks(nc, mask_gp, st, tag):
    """GPSIMD-owned [P,1024] mask units, emitted as early as possible."""
    tiles = st.setdefault("pre_tiles", {})
    for half, jc in sorted(st["gp_set"]):
        mt = mask_gp.tile([P, 1024], BF, tag="mtg",
                          name=f"mtg{tag}_{half}_{jc}")
        nc.gpsimd.tensor_scalar(
            mt, st["s1b"][:, half * 1024:(half + 1) * 1024],
            st["s12"][:, jc, 1:2], 0.0, OP.add, OP.is_ge)
        tiles[(half, jc)] = mt


def _emit_dve_masks(nc, mask_dve, st, units, tag):
    """Pre-emit DVE mask units (fills DVE idle at layer boundaries)."""
    tiles = st.setdefault("pre_tiles", {})
    for half, jc in units:
        if (half, jc) in tiles or (half, jc) in st["gp_set"]:
            continue
        mt = mask_dve.tile([P, 1024], BF, tag="mtd",
                           name=f"mtd{tag}_{half}_{jc}")
        nc.vector.tensor_scalar(mt,
                                st["s1b"][:, half * 1024:(half + 1) * 1024],
                                st["s12"][:, jc, 1:2], 0.0, OP.add, OP.is_ge)
        tiles[(half, jc)] = mt


def _attention(nc, pools, scratch, st, out_cb, onorm_gp, tag, hook=None):
    """Masked-matmul attention core; st holds the layer's prepped tensors."""
    const, prep, mask_dve, mask_gp, wide, small, psA, psaux = pools
    s12, uv = st["s12"], st["uv"]
    g = st["g"]
    ones_row_bf = scratch["ones_row_bf"]

    nsum_w = wide.tile([P, NCH, DEXT], FP, tag="nsum", name=f"nsum_{tag}")
    onorm = wide.tile([P, NCH, D], FP, tag="onorm", name=f"onorm_{tag}")
    LOOKAHEAD = 7
    steps = [(q, jc) for q in range(4) for jc in range(NCH)]
    tiles = dict(st.get("pre_tiles", ()))  # (half, jc) -> [P,1024] tile

    def emit_step(step_idx):
        if step_idx >= len(steps):
            return
        q, jc = steps[step_idx]
        half = q // 2
        if (half, jc) in tiles:
            return
        mt = mask_dve.tile([P, 1024], BF, tag="mtd",
                           name=f"mtd{tag}_{half}_{jc}")
        nc.vector.tensor_scalar(mt, st["s1b"][:, half * 1024:(half + 1) * 1024],
                                s12[:, jc, 1:2], 0.0, OP.add, OP.is_ge)
        tiles[(half, jc)] = mt

    for i in range(LOOKAHEAD):
        emit_step(i)

    for q in range(4):  # quarters of the i (destination-node) axis
        A = [psA.tile([P, 2 * DEXT], FP, tag="A", name=f"A{tag}_{q}_{il}")
             for il in range(4)]
        half, off = q // 2, (q % 2) * 512
        for jc in range(NCH):
            mt = tiles[(half, jc)]
            emit_step(q * NCH + jc + LOOKAHEAD)
            for il in range(4):
                sl = mt[:, off + il * P:off + (il + 1) * P]
                nc.tensor.matmul(A[il], sl, uv[:, jc, :],
                                 start=(jc == 0), stop=False)
        # seed vtot last (PSUM accumulation is order-insensitive): two K=1
        # matmuls (hi + res rows) -- no cross-partition DMA hop needed
        for il in range(4):
            nc.tensor.matmul(A[il], ones_row_bf[0:1, :], st["vrow"],
                             start=False, stop=False)
            nc.tensor.matmul(A[il], ones_row_bf[0:1, :], st["vres"],
                             start=False, stop=True)
        qs = slice(q * 4, (q + 1) * 4)
        for il in range(4):
            ic = q * 4 + il
            # nsum = g * (M@u) + (vtot - M@v): ACT evacuates the u-half with
            # the g-scale fused (Copy+scale), DVE adds the PSUM w-half.
            nU = small.tile([P, DEXT], FP, tag="nU", name=f"nU{tag}_{ic}")
            nc.scalar.activation(nU, A[il][:, 0:DEXT], AF.Copy,
                                 scale=g[:, ic:ic + 1])
            nc.vector.tensor_tensor(nsum_w[:, ic, :], nU, A[il][:, DEXT:],
                                    OP.add)
        rz = small.tile([P, 4], FP, tag="rz", name=f"rz{tag}_{q}")
        nc.vector.reciprocal(rz, nsum_w[:, qs, D])
        for k in range(4):
            ic = q * 4 + k
            if onorm_gp:
                nc.gpsimd.tensor_scalar(onorm[:, ic, :], nsum_w[:, ic, 0:D],
                                        rz[:, k:k + 1], None, OP.mult)
            else:
                nc.scalar.activation(onorm[:, ic, :], nsum_w[:, ic, 0:D],
                                     AF.Copy, scale=rz[:, k:k + 1])
        out_cb(onorm, q)
        if hook is not None:
            hook(q)


def _elu_q(nc, wide, onorm, q, dst_writer, tag):
    """elu over quarter q of onorm [P, NCH, D]; writes via dst_writer(src)."""
    src = onorm[:, q * 4:(q + 1) * 4, :]
    m = wide.tile([P, 4, D], FP, tag="elu_m", name=f"elu_m{tag}{q}")
    nc.gpsimd.tensor_scalar(m, src, 0.0, None, OP.min)
    e = wide.tile([P, 4, D], FP, tag="elu_e", name=f"elu_e{tag}{q}")
    nc.scalar.activation(e, m, AF.Exp)
    r = wide.tile([P, 4, D], FP, tag="elu_r", name=f"elu_r{tag}{q}")
    nc.gpsimd.tensor_scalar(r, src, 0.0, -1.0, OP.max, OP.add)
    dst_writer(r, e)


def build_kernel():
    nc = bacc.Bacc("TRN2", target_bir_lowering=False, debug=False,
                   num_devices=B)

    x = nc.dram_tensor("x", [N, D], FP, kind="ExternalInput")
    W_heads = nc.dram_tensor("W_heads", [H, D, D], FP, kind="ExternalInput")
    a_heads = nc.dram_tensor("a_heads", [H, 2 * D], FP, kind="ExternalInput")
    W_out = nc.dram_tensor("W_out", [H * D, D], FP, kind="ExternalInput")
    a_out = nc.dram_tensor("a_out", [2 * D], FP, kind="ExternalInput")
    out = nc.dram_tensor("out", [N, D], FP, kind="ExternalOutput")

    with tile.TileContext(nc) as tc, ExitStack() as ctx:
        const = ctx.enter_context(tc.tile_pool(name="const", bufs=1))
        prep = ctx.enter_context(tc.tile_pool(name="prep", bufs=3))
        mask_dve = ctx.enter_context(tc.tile_pool(name="mask_dve", bufs=19))
        mask_gp = ctx.enter_context(tc.tile_pool(name="mask_gp", bufs=12))
        wide = ctx.enter_context(tc.tile_pool(name="wide", bufs=2))
        small = ctx.enter_context(tc.tile_pool(name="small", bufs=6))
        psA = ctx.enter_context(tc.tile_pool(name="psA", bufs=6, space="PSUM"))
        psaux = ctx.enter_context(tc.tile_pool(name="psaux", bufs=2,
                                               space="PSUM"))
        pools = (const, prep, mask_dve, mask_gp, wide, small, psA, psaux)

        ident = const.tile([P, P], FP)
        make_identity(nc, ident)
        ones128 = const.tile([P, P], FP)
        nc.vector.memset(ones128, 1.0)
        ones_col_bf = const.tile([P, 1], BF)
        nc.vector.memset(ones_col_bf, 1.0)
        ones_row_bf = const.tile([1, P], BF)
        nc.vector.memset(ones_row_bf, 1.0)
        scratch = {"ones128": ones128, "ones_col_bf": ones_col_bf,
                   "ones_row_bf": ones_row_bf}

        # ---- load inputs: small params first (wa matmuls head PE's program
        # order), then x in 4 pieces so transposes start early ----
        WhT = const.tile([64, H, D], FP)
        nc.sync.dma_start(out=WhT, in_=W_heads.rearrange("h k d -> d h k"))
        a_sb = const.tile([64, H, 2], FP)
        nc.sync.dma_start(out=a_sb, in_=a_heads.rearrange("h (t k) -> k h t", t=2))
        Wh = const.tile([64, H, D], FP)
        nc.sync.dma_start(out=Wh, in_=W_heads.rearrange("h k d -> k h d"))
        x_sb = const.tile([P, NCH, D], FP)
        x_r = x.rearrange("(c p) d -> p c d", p=P)
        for r4 in range(4):
            nc.sync.dma_start(out=x_sb[:, r4 * 4:(r4 + 1) * 4, :],
                              in_=x_r[:, r4 * 4:(r4 + 1) * 4, :])
        Wo = const.tile([P, 2, D], FP)
        nc.sync.dma_start(out=Wo, in_=W_out.rearrange("(c k) d -> k c d", k=P))
        WoT = const.tile([64, 2, P], FP)
        nc.sync.dma_start(out=WoT, in_=W_out.rearrange("(c k) d -> d c k", k=P))
        ao = const.tile([64, 2], FP)
        nc.sync.dma_start(out=ao, in_=a_out.rearrange("(t k) -> k t", t=2))

        # bf16 shadows of the moving matmul operands (4x cheaper PE rows)
        Wh_bf = const.tile([64, H, D], BF)
        nc.vector.tensor_copy(Wh_bf, Wh)
        Wo_bf = const.tile([P, 2, D], BF)
        nc.vector.tensor_copy(Wo_bf, Wo)

        # all heads' wa = W_h @ [a1|a2] upfront (re-association: s = x @ wa);
        # only needs the parameter DMAs, so it fills the startup bubble
        wa_all = const.tile([64, H, 2], FP)
        for h in range(H):
            wap = psaux.tile([64, 2], FP, tag="aux", name=f"wap{h}")
            nc.tensor.matmul(wap, WhT[:, h, :], a_sb[:, h, :], start=True,
                             stop=True)
            nc.scalar.copy(wa_all[:, h, :], wap)

        xT = const.tile([64, N], FP)
        xT_bf = const.tile([64, N], BF)

        def new_state(tag, gp_set):
            st = {"gp_set": gp_set}
            st["s12"] = prep.tile([P, NCH, 2], FP, tag="s12",
                                  name=f"s12_{tag}")
            st["s1b"] = prep.tile([P, N], BF, tag="s1b", name=f"s1b_{tag}")
            st["hext"] = prep.tile([P, NCH, DEXT], BF, tag="hext",
                                   name=f"hext_{tag}")
            nc.vector.memset(st["hext"][:, :, D], 1.0)
            st["uv"] = prep.tile([P, NCH, 2 * DEXT], BF, tag="uv",
                                 name=f"uv_{tag}")
            st["es2"] = prep.tile([P, NCH], FP, tag="es2", name=f"es2_{tag}")
            st["nes2"] = prep.tile([P, NCH], FP, tag="nes2",
                                   name=f"nes2_{tag}")
            st["g"] = prep.tile([P, NCH], FP, tag="g", name=f"g_{tag}")
            return st

        def l1_partA(st, h, cgs=range(4)):
            # s12 columns (batched copies, 4 chunks per PSUM tile)
            wa = wa_all[:, h, :]
            for cg in cgs:
                sp = psaux.tile([P, 8], FP, tag="aux", name=f"sp{h}_{cg}")
                for k in range(4):
                    c = cg * 4 + k
                    nc.tensor.matmul(sp[:, 2 * k:2 * k + 2],
                                     xT[:, c * P:(c + 1) * P], wa,
                                     start=True, stop=True)
                nc.scalar.copy(st["s12"][:, cg * 4:(cg + 1) * 4, :], sp)

        def l1_partB(st, h, rs=range(4), halves=range(2)):
            # s1b (bf16 mask input): s1 row replicated via (ones*wa1) x xT
            wa = wa_all[:, h, :]
            wa1b = prep.tile([64, P], BF, tag="wa1b", name=f"wa1b_{h}")
            nc.vector.tensor_scalar(wa1b, ones128[0:64, :], wa[:, 0:1], None,
                                    OP.mult)
            for r in rs:
                ps = psaux.tile([P, 512], FP, tag="aux", name=f"s1p{h}_{r}")
                nc.tensor.matmul(ps, wa1b, xT_bf[:, r * 512:(r + 1) * 512],
                                 start=True, stop=True)
                nc.scalar.copy(st["s1b"][:, r * 512:(r + 1) * 512], ps)
            # h natural (+ones col), bf16, batched 8 chunks per PSUM bank
            for half in halves:
                hp = psaux.tile([P, 8, D], FP, tag="aux", name=f"hp{h}_{half}")
                for k in range(8):
                    c = half * 8 + k
                    nc.tensor.matmul(hp[:, k, :], xT_bf[:, c * P:(c + 1) * P],
                                     Wh_bf[:, h, :], start=(k == 0),
                                     stop=(k == 7))
                nc.scalar.copy(st["hext"][:, half * 8:(half + 1) * 8, 0:D], hp)

        def l1_partC(st, h):
            _emit_prep_exps(nc, prep, st, range(4))
            nc.scalar.activation(st["g"], st["s12"][:, :, 0], AF.Exp,
                                 scale=1.0 - ALPHA)
            _emit_gp_masks(nc, mask_gp, st, f"h{h}")
            _emit_dve_masks(nc, mask_dve, st,
                            [(0, jc) for jc in range(6)], f"h{h}")
            _emit_uv(nc, st, range(NCH))
            _emit_vrows(nc, prep, psaux, scratch, st, f"h{h}")

        # ---- startup: interleave x transposes with head-0 prep ----
        st0 = new_state("h0", GP_SHORT)
        for r4 in range(4):
            for k in range(4):
                c = r4 * 4 + k
                tp = psaux.tile([64, P], FP, tag="aux", name=f"tp{c}")
                nc.tensor.transpose(tp, x_sb[:, c, :], ident)
                if c % 2 == 0:
                    nc.vector.tensor_copy(xT[:, c * P:(c + 1) * P], tp)
                else:
                    nc.scalar.copy(xT[:, c * P:(c + 1) * P], tp)
            nc.vector.tensor_copy(xT_bf[:, r4 * 512:(r4 + 1) * 512],
                                  xT[:, r4 * 512:(r4 + 1) * 512])
            l1_partA(st0, 0, cgs=[r4])
            l1_partB(st0, 0, rs=[r4], halves=[])
        l1_partB(st0, 0, rs=[], halves=range(2))
        l1_partC(st0, 0)

        # ---- layer 1: four heads -> xc01/xc23 ----
        xc01 = const.tile([P, NCH, 2, D], FP)
        xc23 = const.tile([P, NCH, 2, D], FP)
        xcT_bf = const.tile([P, 2, N], BF)

        # layer-2 state (filled by head-3 hooks)
        st2 = None
        states = [st0]

        def l2_boundary(hq):
            """Pipelined into head 3's quarters: transpose the finished xc
            quarter, then the layer-2 projections that depend on it."""
            st = st2
            for c in range(hq * 4, hq * 4 + 4):
                for kc, xc in ((0, xc01), (1, xc23)):
                    tp = psaux.tile([P, P], FP, tag="aux", name=f"tc{c}_{kc}")
                    nc.tensor.transpose(tp, xc[:, c, :, :], ident)
                    if (c + kc) % 2 == 0:
                        nc.vector.tensor_copy(
                            xcT_bf[:, kc, c * P:(c + 1) * P], tp)
                    else:
                        nc.scalar.copy(xcT_bf[:, kc, c * P:(c + 1) * P], tp)
            if hq == 0:
                # wa2 = W_out @ [a1|a2] (bf16 shadow for the bf16 stationary)
                wa2 = prep.tile([P, 2, 2], FP, tag="wa2")
                for kc in range(2):
                    wap = psaux.tile([P, 2], FP, tag="aux", name=f"wap2_{kc}")
                    nc.tensor.matmul(wap, WoT[:, kc, :], ao, start=True,
                                     stop=True)
                    nc.scalar.copy(wa2[:, kc, :], wap)
                wa2b = prep.tile([P, 2, 2], BF, tag="wa2b")
                nc.vector.tensor_copy(wa2b, wa2)
                st["wa2b"] = wa2b
                wa1b2 = prep.tile([P, 2, P], BF, tag="wa1b2")
                for kc in range(2):
                    nc.vector.tensor_scalar(wa1b2[:, kc, :], ones128,
                                            wa2[:, kc, 0:1], None, OP.mult)
                st["wa1b2"] = wa1b2
            # s12_2 for this chunk group
            cg = hq
            sp = psaux.tile([P, 8], FP, tag="aux", name=f"sp2_{cg}")
            for k in range(4):
                c = cg * 4 + k
                for kc in range(2):
                    nc.tensor.matmul(sp[:, 2 * k:2 * k + 2],
                                     xcT_bf[:, kc, c * P:(c + 1) * P],
                                     st["wa2b"][:, kc, :],
                                     start=(kc == 0), stop=(kc == 1))
            nc.scalar.copy(st["s12"][:, cg * 4:(cg + 1) * 4, :], sp)
            _emit_prep_exps(nc, prep, st, [cg])
            # s1b_2 piece hq (needs xcT chunks 4hq..4hq+3)
            r = hq
            ps = psaux.tile([P, 512], FP, tag="aux", name=f"s1p2_{r}")
            for kc in range(2):
                nc.tensor.matmul(ps, st["wa1b2"][:, kc, :],
                                 xcT_bf[:, kc, r * 512:(r + 1) * 512],
                                 start=(kc == 0), stop=(kc == 1))
            nc.scalar.copy(st["s1b"][:, r * 512:(r + 1) * 512], ps)
            # h2ext halves once their 8 chunks are transposed
            if hq in (1, 3):
                half = hq // 2
                hp = psaux.tile([P, 8, D], FP, tag="aux", name=f"hp2_{half}")
                for k in range(8):
                    c = half * 8 + k
                    for kc in range(2):
                        nc.tensor.matmul(hp[:, k, :],
                                         xcT_bf[:, kc, c * P:(c + 1) * P],
                                         Wo_bf[:, kc, :],
                                         start=(k == 0 and kc == 0),
                                         stop=(k == 7 and kc == 1))
                nc.scalar.copy(st["hext"][:, half * 8:(half + 1) * 8, 0:D],
                               hp)
                _emit_uv(nc, st, range(half * 8, half * 8 + 8))
            if hq == 2:
                # s12 cg0/1 and s1b r0/1 are ready -> pre-make early masks
                _emit_dve_masks(nc, mask_dve, st,
                                [(0, jc) for jc in range(6)], "l2")
            if hq == 3:
                nc.scalar.activation(st["g"], st["s12"][:, :, 0], AF.Exp,
                                     scale=1.0 - ALPHA)
                _emit_gp_masks(nc, mask_gp, st, "l2")
                _emit_dve_masks(nc, mask_dve, st,
                                [(0, jc) for jc in range(6, 12)], "l2")
                _emit_vrows(nc, prep, psaux, scratch, st, "l2")

        for h in range(H):
            st = states[h]

            def l1_out(onorm, q, h=h):
                xc = xc01 if h < 2 else xc23

                def write(r, e):
                    nc.vector.tensor_tensor(
                        xc[:, q * 4:(q + 1) * 4, h % 2, :], r, e, OP.add)

                _elu_q(nc, wide, onorm, q, write, f"h{h}")

            if h < H - 1:
                nst = new_state(f"h{h + 1}", GP_MAIN)
                states.append(nst)

                def hook(q, h=h, nst=nst):
                    if q == 1:
                        l1_partA(nst, h + 1)
                    elif q == 2:
                        l1_partB(nst, h + 1)
                    elif q == 3:
                        l1_partC(nst, h + 1)
            else:
                st2 = new_state("l2", GP_SHORT)

                def hook(q):
                    l2_boundary(q)

            _attention(nc, pools, scratch, st, l1_out, False, f"h{h}", hook)

        # ---- layer 2 attention + elu + per-quarter log_softmax -> out ----
        out_w = const.tile([P, NCH, D], FP)
        out_r = out.rearrange("(c p) d -> p c d", p=P)
        o2_all = const.tile([P, NCH, D], FP)
        esum_all = const.tile([P, NCH], FP)
        lse = const.tile([P, NCH], FP)

        def l2_out(onorm, q):
            # elu + raw exp-sum (elu output is <= ~20, so exp is fp32-safe
            # without max subtraction); Ln + final subtract deferred so the
            # Exp/Ln ACT tables swap once, not per quarter
            qs = slice(q * 4, (q + 1) * 4)
            o2 = o2_all[:, qs, :]

            def write(r, e):
                nc.vector.tensor_tensor(o2, r, e, OP.add)

            _elu_q(nc, wide, onorm, q, write, "l2")
            escr = wide.tile([P, 4, D], FP, tag="escr", name=f"escr{q}")
            for k in range(4):
                ic = q * 4 + k
                nc.scalar.activation(escr[:, k, :], o2[:, k, :], AF.Exp,
                                     accum_out=esum_all[:, ic:ic + 1])

        _attention(nc, pools, scratch, st2, l2_out, True, "l2")

        nc.scalar.activation(lse, esum_all, AF.Ln)
        lse_b = bass.AP(tensor=lse.tensor, offset=lse.offset,
                        ap=[lse.ap[0], lse.ap[1], [0, D]])
        nc.vector.tensor_tensor(out_w, o2_all, lse_b, OP.subtract)
        nc.sync.dma_start(out=out_r, in_=out_w)

    nc.compile()
    return nc


_NC_CACHE = {}


def _make_runner(nc):
    """Build a cached sharded executable (run_bass_kernel_spmd re-traces
    jax.jit on every call; this jits once and reuses)."""
    import jax
    from jax.sharding import Mesh, PartitionSpec
    try:
        from jax.experimental.shard_map import shard_map
    except ImportError:
        from jax.shard_map import shard_map
    import concourse.mybir as mb
    from concourse import bass2jax

    bass2jax.install_neuronx_cc_hook()

    part_name = nc.partition_id_tensor.name if nc.partition_id_tensor else None
    in_names, out_names, out_avals = [], [], []
    for alloc in nc.m.functions[0].allocations:
        if not isinstance(alloc, mb.MemoryLocationSet):
            continue
        name = alloc.memorylocations[0].name
        if alloc.kind == "ExternalInput":
            if name != part_name:
                in_names.append(name)
        elif alloc.kind == "ExternalOutput":
            out_names.append(name)
            out_avals.append(jax.core.ShapedArray(
                tuple(alloc.tensor_shape), mb.dt.np(alloc.dtype)))
    n_params = len(in_names)
    all_names = in_names + out_names
    if part_name is not None:
        all_names = all_names + [part_name]

    def _body(*args):
        operands = list(args)
        if part_name is not None:
            operands.append(bass2jax.partition_id_tensor())
        return tuple(bass2jax._bass_exec_p.bind(
            *operands, out_avals=tuple(out_avals), in_names=tuple(all_names),
            out_names=tuple(out_names), lowering_input_output_aliases=(),
            sim_require_finite=True, sim_require_nnan=True, nc=nc))

    devices = jax.devices()[:B]
    mesh = Mesh(np.asarray(devices), ("core",))
    n_outs = len(out_names)
    sharded = jax.jit(
        shard_map(_body, mesh=mesh,
                  in_specs=(PartitionSpec("core"),) * (n_params + n_outs),
                  out_specs=(PartitionSpec("core"),) * n_outs,
                  check_rep=False),
        donate_argnums=tuple(range(n_params, n_params + n_outs)),
        keep_unused=True)

    def run(in_maps):
        concat_in = [
            np.concatenate([np.asarray(in_maps[c][nm])[None] for c in range(B)],
                           axis=0).reshape(B * in_maps[0][nm].shape[0],
                                           *in_maps[0][nm].shape[1:])
            for nm in in_names
        ]
        concat_zeros = [
            np.zeros((B * av.shape[0], *av.shape[1:]), av.dtype)
            for av in out_avals
        ]
        out_arrs = sharded(*concat_in, *concat_zeros)
        return [
            {nm: np.asarray(out_arrs[i]).reshape(B, *out_avals[i].shape)[c]
             for i, nm in enumerate(out_names)}
            for c in range(B)
        ]

    return run


def kernel(**inputs):
    h_states = np.ascontiguousarray(np.asarray(inputs["h_states"], dtype=np.float32))
    W_heads = np.ascontiguousarray(np.asarray(inputs["W_heads"], dtype=np.float32))
    a_heads = np.ascontiguousarray(np.asarray(inputs["a_heads"], dtype=np.float32))
    W_out = np.ascontiguousarray(np.asarray(inputs["W_out"], dtype=np.float32))
    a_out = np.ascontiguousarray(np.asarray(inputs["a_out"], dtype=np.float32))

    if "nc" not in _NC_CACHE:
        _NC_CACHE["nc"] = build_kernel()
        _NC_CACHE["run"] = _make_runner(_NC_CACHE["nc"])

    xs = h_states.reshape(B, N, D)
    in_maps = [
        {"x": xs[c], "W_heads": W_heads, "a_heads": a_heads,
         "W_out": W_out, "a_out": a_out}
        for c in range(B)
    ]
    results = _NC_CACHE["run"](in_maps)
    return np.concatenate([results[c]["out"] for c in range(B)], axis=0)


if __name__ == "__main__":
    # smoke test (self-contained: random inputs, shape/dtype check only)
    rng = np.random.default_rng(0)
    inputs = {
        "h_states": rng.standard_normal((B * N, D)).astype(np.float32),
        "W_heads": rng.standard_normal((H, D, D)).astype(np.float32) * 0.18,
        "a_heads": rng.standard_normal((H, 2 * D)).astype(np.float32) * 0.18,
        "W_out": rng.standard_normal((H * D, D)).astype(np.float32) * 0.09,
        "a_out": rng.standard_normal((2 * D,)).astype(np.float32) * 0.18,
        "seq_start_end": (np.arange(B, dtype=np.int32)[:, None] * N
                          + np.array([0, N], dtype=np.int32)[None, :]),
    }
    got = kernel(**inputs)
    print("kernel output", got.shape, got.dtype)


# revision 43
# speedup vs baseline: 1.0565x; 1.0202x over previous
"""Bass/Tile Trainium2 kernel for a 2-layer dense multi-head GAT over a batch
of B=8 independent subgraphs (2048 nodes each, equal contiguous segments).

Sharding: one subgraph per NeuronCore (8 cores), parameters replicated.

Algorithm (per core / subgraph, per attention layer):
  scores are rank-1:  e_ij = leaky_relu(s1_i + s2_j),  s1 = h@a1, s2 = h@a2.
  exp(leaky_relu(t)) is separable through the sign mask M_ij = [s1_i+s2_j>=0]:
      p_ij = M_ij e^{s1_i} e^{s2_j} + (1-M_ij) e^{a s1_i} e^{a s2_j}
  so softmax(e) @ h needs NO N^2 exp work:
      num_i = g_i (M @ u)_i + (vtot - (M @ v))_i          (e^{a s1} cancels in
      u_j = e^{s2_j} [h_j|1],  v_j = e^{a s2_j} [h_j|1],   the Z ratio; g =
      out_i = num_i[:64] / num_i[64]                       e^{(1-a) s1})
  The N^2 work is the 0/1 mask build (exact in bf16; DVE is_ge in its 2x
  bf16 mode, with the tail jc columns on GPSIMD) plus mask matmuls with a
  single bf16 [u|-v] stream.  Each head's projections/exponentials are
  emitted inside the previous head's attention quarters so every engine
  stays busy across head boundaries; the layer-2 transposes/projections are
  likewise pipelined into head 3's quarters, and the log_softmax tail is
  per-quarter so the output DMA overlaps the last attention.
"""

from contextlib import ExitStack

import numpy as np

import concourse.bass as bass
import concourse.tile as tile
from concourse import bacc, mybir
from concourse.masks import make_identity

FP = mybir.dt.float32
BF = mybir.dt.bfloat16
AF = mybir.ActivationFunctionType
OP = mybir.AluOpType

B = 8
N = 2048
D = 64
H = 4
ALPHA = 0.2
P = 128
NCH = N // P  # 16 chunks of 128 nodes
DEXT = D + 1  # h plus ones column

# mask ownership: DVE makes every mask tile (GPSIMD is 4.2x slower per mask
# but only ~1.5x slower on elementwise work, so GPSIMD gets uv/onorm/elu and
# DVE gets all masks -- the total-work-minimizing split).
GP_MAIN = frozenset()
GP_SHORT = frozenset()


def _emit_prep_exps(nc, prep, st, cgs):
    """es2/nes2 exponentials for s12 column groups cgs (4 cols each)."""
    for cg in cgs:
        gs = slice(cg * 4, (cg + 1) * 4)
        nc.scalar.activation(st["es2"][:, gs], st["s12"][:, gs, 1], AF.Exp)
        nc.scalar.activation(st["nes2"][:, gs], st["s12"][:, gs, 1], AF.Exp,
                             scale=ALPHA)


def _emit_uv(nc, st, chunks):
    """u | -v bf16 stream for node chunks (the -1 rides the 2nd ALU op)."""
    uv, hext = st["uv"], st["hext"]
    for c in chunks:
        nc.gpsimd.tensor_scalar(uv[:, c, 0:DEXT], hext[:, c, :],
                                st["es2"][:, c:c + 1], None, OP.mult)
        nc.gpsimd.tensor_scalar(uv[:, c, DEXT:], hext[:, c, :],
                                st["nes2"][:, c:c + 1], -1.0, OP.mult, OP.mult)


def _emit_vrows(nc, prep, psaux, scratch, st, tag):
    """vtot = sum_j v_j via PE; [2, 130] bf16 hi+res seed rows assembled via
    a DMA hop (compute engines cannot write partition 1; the hop is off the
    critical path since it lands during the previous head's quarters)."""
    vt_ps = psaux.tile([1, DEXT], FP, tag="aux", name=f"vt_{tag}")
    for c in range(NCH):
        nc.tensor.matmul(vt_ps, scratch["ones_col_bf"], st["uv"][:, c, DEXT:],
                         start=(c == 0), stop=(c == NCH - 1))
    vrow = prep.tile([1, 2 * DEXT], BF, tag="vrow", name=f"vrow_{tag}")
    nc.vector.memset(vrow[:, 0:DEXT], 0.0)
    nc.vector.tensor_scalar(vrow[:, DEXT:], vt_ps, -1.0, None, OP.mult)
    vres = prep.tile([1, DEXT], BF, tag="vres", name=f"vres_{tag}")
    nc.vector.scalar_tensor_tensor(vres, vt_ps, -1.0, vrow[:, DEXT:],
                                   OP.mult, OP.subtract)
    vrow2 = prep.tile([2, 2 * DEXT], BF, tag="vrow2", name=f"vrow2_{tag}")
    nc.sync.dma_start(out=vrow2[0:1, :], in_=vrow)
    nc.sync.dma_start(out=vrow2[1:2, 0:DEXT], in_=vrow[:, 0:DEXT])
    nc.sync.dma_start(out=vrow2[1:2, DEXT:], in_=vres)
    st["vrow2"] = vrow2


def _emit_gp_masks(nc, mask_gp, st, tag):
    """GPSIMD-owned [P,1024] mask units, emitted as early as possible."""
    tiles = st.setdefault("pre_tiles", {})
    for half, jc in sorted(st["gp_set"]):
        mt = mask_gp.tile([P, 1024], BF, tag="mtg",
                          name=f"mtg{tag}_{half}_{jc}")
        nc.gpsimd.tensor_scalar(
            mt, st["s1b"][:, half * 1024:(half + 1) * 1024],
            st["s12"][:, jc, 1:2], 0.0, OP.add, OP.is_ge)
        tiles[(half, jc)] = mt


def _emit_dve_masks(nc, mask_dve, st, units, tag):
    """Pre-emit DVE mask units (fills DVE idle at layer boundaries)."""
    tiles = st.setdefault("pre_tiles", {})
    for half, jc in units:
        if (half, jc) in tiles or (half, jc) in st["gp_set"]:
            continue
        mt = mask_dve.tile([P, 1024], BF, tag="mtd",
                           name=f"mtd{tag}_{half}_{jc}")
        nc.vector.tensor_scalar(mt,
                                st["s1b"][:, half * 1024:(half + 1) * 1024],
                                st["s12"][:, jc, 1:2], 0.0, OP.add, OP.is_ge)
        tiles[(half, jc)] = mt


def _attention(nc, pools, scratch, st, out_cb, tag, hook=None,
               hook_first=False):
    """Masked-matmul attention core; st holds the layer's prepped tensors."""
    const, prep, mask_dve, mask_gp, wide, small, psA, psaux = pools
    s12, uv = st["s12"], st["uv"]
    g = st["g"]
    ones_row_bf = scratch["ones_row_bf"]

    nsum_w = wide.tile([P, NCH, DEXT], FP, tag="nsum", name=f"nsum_{tag}")
    onorm = wide.tile([P, NCH, D], FP, tag="onorm", name=f"onorm_{tag}")
    LOOKAHEAD = 7
    steps = [(q, jc) for q in range(4) for jc in range(NCH)]
    tiles = dict(st.get("pre_tiles", ()))  # (half, jc) -> [P,1024] tile

    def emit_step(step_idx):
        if step_idx >= len(steps):
            return
        q, jc = steps[step_idx]
        half = q // 2
        if (half, jc) in tiles:
            return
        mt = mask_dve.tile([P, 1024], BF, tag="mtd",
                           name=f"mtd{tag}_{half}_{jc}")
        nc.vector.tensor_scalar(mt, st["s1b"][:, half * 1024:(half + 1) * 1024],
                                s12[:, jc, 1:2], 0.0, OP.add, OP.is_ge)
        tiles[(half, jc)] = mt

    for i in range(LOOKAHEAD):
        emit_step(i)

    for q in range(4):  # quarters of the i (destination-node) axis
        A = [psA.tile([P, 2 * DEXT], FP, tag="A", name=f"A{tag}_{q}_{il}")
             for il in range(4)]
        half, off = q // 2, (q % 2) * 512
        for jc in range(NCH):
            mt = tiles[(half, jc)]
            emit_step(q * NCH + jc + LOOKAHEAD)
            for il in range(4):
                sl = mt[:, off + il * P:off + (il + 1) * P]
                nc.tensor.matmul(A[il], sl, uv[:, jc, :],
                                 start=(jc == 0), stop=False)
        # seed vtot last (PSUM accumulation is order-insensitive): one K=2
        # matmul adds the bf16 hi+res rows together
        for il in range(4):
            nc.tensor.matmul(A[il], ones_row_bf, st["vrow2"],
                             start=False, stop=True)
        qs = slice(q * 4, (q + 1) * 4)
        for il in range(4):
            ic = q * 4 + il
            # nsum = g * (M@u) + (vtot - M@v): ACT evacuates the u-half with
            # the g-scale fused (Copy+scale), DVE adds the PSUM w-half.
            nU = small.tile([P, DEXT], FP, tag="nU", name=f"nU{tag}_{ic}")
            nc.scalar.activation(nU, A[il][:, 0:DEXT], AF.Copy,
                                 scale=g[:, ic:ic + 1])
            nc.vector.tensor_tensor(nsum_w[:, ic, :], nU, A[il][:, DEXT:],
                                    OP.add)
        rz = small.tile([P, 4], FP, tag="rz", name=f"rz{tag}_{q}")
        nc.vector.reciprocal(rz, nsum_w[:, qs, D])
        for k in range(4):
            ic = q * 4 + k
            nc.gpsimd.tensor_scalar(onorm[:, ic, :], nsum_w[:, ic, 0:D],
                                    rz[:, k:k + 1], None, OP.mult)
        if hook is not None and hook_first:
            hook(q)
        out_cb(onorm, q)
        if hook is not None and not hook_first:
            hook(q)


def _elu_q(nc, wide, onorm, q, dst_writer, tag):
    """elu over quarter q of onorm [P, NCH, D]; writes via dst_writer(src)."""
    src = onorm[:, q * 4:(q + 1) * 4, :]
    m = wide.tile([P, 4, D], FP, tag="elu_m", name=f"elu_m{tag}{q}")
    nc.gpsimd.tensor_scalar(m, src, 0.0, None, OP.min)
    e = wide.tile([P, 4, D], FP, tag="elu_e", name=f"elu_e{tag}{q}")
    nc.scalar.activation(e, m, AF.Exp)
    r = wide.tile([P, 4, D], FP, tag="elu_r", name=f"elu_r{tag}{q}")
    nc.gpsimd.tensor_scalar(r, src, 0.0, -1.0, OP.max, OP.add)
    dst_writer(r, e)


def build_kernel():
    nc = bacc.Bacc("TRN2", target_bir_lowering=False, debug=False,
                   num_devices=B)

    x = nc.dram_tensor("x", [N, D], FP, kind="ExternalInput")
    W_heads = nc.dram_tensor("W_heads", [H, D, D], FP, kind="ExternalInput")
    a_heads = nc.dram_tensor("a_heads", [H, 2 * D], FP, kind="ExternalInput")
    W_out = nc.dram_tensor("W_out", [H * D, D], FP, kind="ExternalInput")
    a_out = nc.dram_tensor("a_out", [2 * D], FP, kind="ExternalInput")
    out = nc.dram_tensor("out", [N, D], FP, kind="ExternalOutput")

    with tile.TileContext(nc) as tc, ExitStack() as ctx:
        const = ctx.enter_context(tc.tile_pool(name="const", bufs=1))
        prep = ctx.enter_context(tc.tile_pool(name="prep", bufs=3))
        mask_dve = ctx.enter_context(tc.tile_pool(name="mask_dve", bufs=26))
        mask_gp = ctx.enter_context(tc.tile_pool(name="mask_gp", bufs=2))
        wide = ctx.enter_context(tc.tile_pool(name="wide", bufs=2))
        small = ctx.enter_context(tc.tile_pool(name="small", bufs=6))
        psA = ctx.enter_context(tc.tile_pool(name="psA", bufs=6, space="PSUM"))
        psaux = ctx.enter_context(tc.tile_pool(name="psaux", bufs=2,
                                               space="PSUM"))
        pools = (const, prep, mask_dve, mask_gp, wide, small, psA, psaux)

        ident = const.tile([P, P], FP)
        make_identity(nc, ident)
        ones128 = const.tile([P, P], FP)
        nc.vector.memset(ones128, 1.0)
        ones_col_bf = const.tile([P, 1], BF)
        nc.vector.memset(ones_col_bf, 1.0)
        ones_row_bf = const.tile([2, P], BF)
        nc.vector.memset(ones_row_bf, 1.0)
        scratch = {"ones128": ones128, "ones_col_bf": ones_col_bf,
                   "ones_row_bf": ones_row_bf}

        # ---- load inputs: small params first (wa matmuls head PE's program
        # order), then x in 4 pieces so transposes start early ----
        WhT = const.tile([64, H, D], FP)
        nc.sync.dma_start(out=WhT, in_=W_heads.rearrange("h k d -> d h k"))
        a_sb = const.tile([64, H, 2], FP)
        nc.sync.dma_start(out=a_sb, in_=a_heads.rearrange("h (t k) -> k h t", t=2))
        Wh = const.tile([64, H, D], FP)
        nc.sync.dma_start(out=Wh, in_=W_heads.rearrange("h k d -> k h d"))
        x_sb = const.tile([P, NCH, D], FP)
        x_r = x.rearrange("(c p) d -> p c d", p=P)
        for r4 in range(4):
            nc.sync.dma_start(out=x_sb[:, r4 * 4:(r4 + 1) * 4, :],
                              in_=x_r[:, r4 * 4:(r4 + 1) * 4, :])
        Wo = const.tile([P, 2, D], FP)
        nc.sync.dma_start(out=Wo, in_=W_out.rearrange("(c k) d -> k c d", k=P))
        WoT = const.tile([64, 2, P], FP)
        nc.sync.dma_start(out=WoT, in_=W_out.rearrange("(c k) d -> d c k", k=P))
        ao = const.tile([64, 2], FP)
        nc.sync.dma_start(out=ao, in_=a_out.rearrange("(t k) -> k t", t=2))

        # bf16 shadows of the moving matmul operands (4x cheaper PE rows)
        Wh_bf = const.tile([64, H, D], BF)
        nc.vector.tensor_copy(Wh_bf, Wh)
        Wo_bf = const.tile([P, 2, D], BF)
        nc.vector.tensor_copy(Wo_bf, Wo)

        # all heads' wa = W_h @ [a1|a2] upfront (re-association: s = x @ wa);
        # only needs the parameter DMAs, so it fills the startup bubble
        wa_all = const.tile([64, H, 2], FP)
        for h in range(H):
            wap = psaux.tile([64, 2], FP, tag="aux", name=f"wap{h}")
            nc.tensor.matmul(wap, WhT[:, h, :], a_sb[:, h, :], start=True,
                             stop=True)
            nc.scalar.copy(wa_all[:, h, :], wap)

        xT = const.tile([64, N], FP)
        xT_bf = const.tile([64, N], BF)

        def new_state(tag, gp_set):
            st = {"gp_set": gp_set}
            st["s12"] = prep.tile([P, NCH, 2], FP, tag="s12",
                                  name=f"s12_{tag}")
            st["s1b"] = prep.tile([P, N], BF, tag="s1b", name=f"s1b_{tag}")
            st["hext"] = prep.tile([P, NCH, DEXT], BF, tag="hext",
                                   name=f"hext_{tag}")
            nc.vector.memset(st["hext"][:, :, D], 1.0)
            st["uv"] = prep.tile([P, NCH, 2 * DEXT], BF, tag="uv",
                                 name=f"uv_{tag}")
            st["es2"] = prep.tile([P, NCH], FP, tag="es2", name=f"es2_{tag}")
            st["nes2"] = prep.tile([P, NCH], FP, tag="nes2",
                                   name=f"nes2_{tag}")
            st["g"] = prep.tile([P, NCH], FP, tag="g", name=f"g_{tag}")
            return st

        def l1_partA(st, h, cgs=range(4)):
            # s12 columns (batched copies, 4 chunks per PSUM tile)
            wa = wa_all[:, h, :]
            for cg in cgs:
                sp = psaux.tile([P, 8], FP, tag="aux", name=f"sp{h}_{cg}")
                for k in range(4):
                    c = cg * 4 + k
                    nc.tensor.matmul(sp[:, 2 * k:2 * k + 2],
                                     xT[:, c * P:(c + 1) * P], wa,
                                     start=True, stop=True)
                nc.scalar.copy(st["s12"][:, cg * 4:(cg + 1) * 4, :], sp)

        def l1_partB(st, h, rs=range(4), halves=range(2), premask=False):
            # s1b (bf16 mask input): s1 row replicated via (ones*wa1) x xT
            wa = wa_all[:, h, :]
            wa1b = prep.tile([64, P], BF, tag="wa1b", name=f"wa1b_{h}")
            nc.vector.tensor_scalar(wa1b, ones128[0:64, :], wa[:, 0:1], None,
                                    OP.mult)
            for r in rs:
                ps = psaux.tile([P, 512], FP, tag="aux", name=f"s1p{h}_{r}")
                nc.tensor.matmul(ps, wa1b, xT_bf[:, r * 512:(r + 1) * 512],
                                 start=True, stop=True)
                nc.scalar.copy(st["s1b"][:, r * 512:(r + 1) * 512], ps)
            # h natural (+ones col), bf16, batched 8 chunks per PSUM bank
            for half in halves:
                hp = psaux.tile([P, 8, D], FP, tag="aux", name=f"hp{h}_{half}")
                for k in range(8):
                    c = half * 8 + k
                    nc.tensor.matmul(hp[:, k, :], xT_bf[:, c * P:(c + 1) * P],
                                     Wh_bf[:, h, :], start=(k == 0),
                                     stop=(k == 7))
                nc.scalar.copy(st["hext"][:, half * 8:(half + 1) * 8, 0:D], hp)
            if premask:
                _emit_dve_masks(nc, mask_dve, st,
                                [(0, jc) for jc in range(8)], f"h{h}")

        def l1_partC(st, h):
            _emit_prep_exps(nc, prep, st, range(4))
            nc.scalar.activation(st["g"], st["s12"][:, :, 0], AF.Exp,
                                 scale=1.0 - ALPHA)
            _emit_uv(nc, st, range(NCH))
            _emit_dve_masks(nc, mask_dve, st,
                            [(0, jc) for jc in range(8, 12)], f"h{h}")
            _emit_vrows(nc, prep, psaux, scratch, st, f"h{h}")

        # ---- startup: interleave x transposes with head-0 prep ----
        st0 = new_state("h0", GP_SHORT)
        for r4 in range(4):
            for k in range(4):
                c = r4 * 4 + k
                tp = psaux.tile([64, P], FP, tag="aux", name=f"tp{c}")
                nc.tensor.transpose(tp, x_sb[:, c, :], ident)
                if c % 2 == 0:
                    nc.vector.tensor_copy(xT[:, c * P:(c + 1) * P], tp)
                else:
                    nc.scalar.copy(xT[:, c * P:(c + 1) * P], tp)
            nc.vector.tensor_copy(xT_bf[:, r4 * 512:(r4 + 1) * 512],
                                  xT[:, r4 * 512:(r4 + 1) * 512])
            l1_partA(st0, 0, cgs=[r4])
            l1_partB(st0, 0, rs=[r4], halves=[])
        l1_partB(st0, 0, rs=[], halves=range(2), premask=True)
        l1_partC(st0, 0)

        # ---- layer 1: four heads -> xc01/xc23 (bf16: halves the transpose
        # row cost and the evac copies run in DVE 2x mode) ----
        xc01 = const.tile([P, NCH, 2, D], BF)
        xc23 = const.tile([P, NCH, 2, D], BF)
        xcT_bf = const.tile([P, 2, N], BF)
        ident_bf = const.tile([P, P], BF)
        nc.vector.tensor_copy(ident_bf, ident)

        # layer-2 state (filled by head-3 hooks)
        st2 = None
        states = [st0]

        def l2_boundary(hq):
            """Pipelined into head 3's quarters: transpose the finished xc
            quarter, then the layer-2 projections that depend on it."""
            st = st2
            for c in range(hq * 4, hq * 4 + 4):
                for kc, xc in ((0, xc01), (1, xc23)):
                    tp = psaux.tile([P, P], BF, tag="aux", name=f"tc{c}_{kc}")
                    nc.tensor.transpose(tp, xc[:, c, :, :], ident_bf)
                    if (c + kc) % 2 == 0:
                        nc.vector.tensor_copy(
                            xcT_bf[:, kc, c * P:(c + 1) * P], tp)
                    else:
                        nc.scalar.copy(xcT_bf[:, kc, c * P:(c + 1) * P], tp)
            if hq == 0:
                # wa2 = W_out @ [a1|a2] (bf16 shadow for the bf16 stationary)
                wa2 = prep.tile([P, 2, 2], FP, tag="wa2")
                for kc in range(2):
                    wap = psaux.tile([P, 2], FP, tag="aux", name=f"wap2_{kc}")
                    nc.tensor.matmul(wap, WoT[:, kc, :], ao, start=True,
                                     stop=True)
                    nc.scalar.copy(wa2[:, kc, :], wap)
                wa2b = prep.tile([P, 2, 2], BF, tag="wa2b")
                nc.vector.tensor_copy(wa2b, wa2)
                st["wa2b"] = wa2b
                wa1b2 = prep.tile([P, 2, P], BF, tag="wa1b2")
                for kc in range(2):
                    nc.vector.tensor_scalar(wa1b2[:, kc, :], ones128,
                                            wa2[:, kc, 0:1], None, OP.mult)
                st["wa1b2"] = wa1b2
            # s12_2 for this chunk group
            cg = hq
            sp = psaux.tile([P, 8], FP, tag="aux", name=f"sp2_{cg}")
            for k in range(4):
                c = cg * 4 + k
                for kc in range(2):
                    nc.tensor.matmul(sp[:, 2 * k:2 * k + 2],
                                     xcT_bf[:, kc, c * P:(c + 1) * P],
                                     st["wa2b"][:, kc, :],
                                     start=(kc == 0), stop=(kc == 1))
            nc.scalar.copy(st["s12"][:, cg * 4:(cg + 1) * 4, :], sp)
            _emit_prep_exps(nc, prep, st, [cg])
            # s1b_2 piece hq (needs xcT chunks 4hq..4hq+3)
            r = hq
            ps = psaux.tile([P, 512], FP, tag="aux", name=f"s1p2_{r}")
            for kc in range(2):
                nc.tensor.matmul(ps, st["wa1b2"][:, kc, :],
                                 xcT_bf[:, kc, r * 512:(r + 1) * 512],
                                 start=(kc == 0), stop=(kc == 1))
            nc.scalar.copy(st["s1b"][:, r * 512:(r + 1) * 512], ps)
            # h2ext halves once their 8 chunks are transposed
            if hq in (1, 3):
                half = hq // 2
                hp = psaux.tile([P, 8, D], FP, tag="aux", name=f"hp2_{half}")
                for k in range(8):
                    c = half * 8 + k
                    for kc in range(2):
                        nc.tensor.matmul(hp[:, k, :],
                                         xcT_bf[:, kc, c * P:(c + 1) * P],
                                         Wo_bf[:, kc, :],
                                         start=(k == 0 and kc == 0),
                                         stop=(k == 7 and kc == 1))
                nc.scalar.copy(st["hext"][:, half * 8:(half + 1) * 8, 0:D],
                               hp)
                _emit_uv(nc, st, range(half * 8, half * 8 + 8))
            if hq == 2:
                # s12 cg0/1 and s1b r0/1 are ready -> pre-make early masks
                _emit_dve_masks(nc, mask_dve, st,
                                [(0, jc) for jc in range(6)], "l2")
            if hq == 3:
                nc.scalar.activation(st["g"], st["s12"][:, :, 0], AF.Exp,
                                     scale=1.0 - ALPHA)
                _emit_dve_masks(nc, mask_dve, st,
                                [(0, jc) for jc in range(6, 12)], "l2")
                _emit_vrows(nc, prep, psaux, scratch, st, "l2")

        for h in range(H):
            st = states[h]

            def l1_out(onorm, q, h=h):
                xc = xc01 if h < 2 else xc23

                def write(r, e):
                    nc.gpsimd.tensor_tensor(
                        xc[:, q * 4:(q + 1) * 4, h % 2, :], r, e, OP.add)

                _elu_q(nc, wide, onorm, q, write, f"h{h}")

            if h < H - 1:
                nst = new_state(f"h{h + 1}", GP_MAIN)
                states.append(nst)

                def hook(q, h=h, nst=nst):
                    if q == 1:
                        l1_partA(nst, h + 1)
                    elif q == 2:
                        l1_partB(nst, h + 1, premask=True)
                    elif q == 3:
                        l1_partC(nst, h + 1)
            else:
                st2 = new_state("l2", GP_SHORT)

                def hook(q):
                    l2_boundary(q)

            _attention(nc, pools, scratch, st, l1_out, f"h{h}", hook,
                       hook_first=(h < H - 1))

        # ---- layer 2 attention + elu + per-quarter log_softmax -> out ----
        out_w = const.tile([P, NCH, D], FP)
        out_r = out.rearrange("(c p) d -> p c d", p=P)
        o2_all = const.tile([P, NCH, D], FP)
        esum_all = const.tile([P, NCH], FP)
        lse = const.tile([P, NCH], FP)

        def l2_out(onorm, q):
            # elu + raw exp-sum (elu output is <= ~20, so exp is fp32-safe
            # without max subtraction); Ln + final subtract deferred so the
            # Exp/Ln ACT tables swap once, not per quarter
            qs = slice(q * 4, (q + 1) * 4)
            o2 = o2_all[:, qs, :]

            def write(r, e):
                nc.gpsimd.tensor_tensor(o2, r, e, OP.add)

            _elu_q(nc, wide, onorm, q, write, "l2")
            escr = wide.tile([P, 4, D], FP, tag="escr", name=f"escr{q}")
            for k in range(4):
                ic = q * 4 + k
                nc.scalar.activation(escr[:, k, :], o2[:, k, :], AF.Exp,
                                     accum_out=esum_all[:, ic:ic + 1])

        _attention(nc, pools, scratch, st2, l2_out, "l2")

        nc.scalar.activation(lse, esum_all, AF.Ln)
        lse_b = bass.AP(tensor=lse.tensor, offset=lse.offset,
                        ap=[lse.ap[0], lse.ap[1], [0, D]])
        nc.vector.tensor_tensor(out_w, o2_all, lse_b, OP.subtract)
        nc.sync.dma_start(out=out_r, in_=out_w)

    nc.compile()
    return nc


_NC_CACHE = {}


def _make_runner(nc):
    """Build a cached sharded executable (run_bass_kernel_spmd re-traces
    jax.jit on every call; this jits once and reuses)."""
    import jax
    from jax.sharding import Mesh, PartitionSpec
    try:
        from jax.experimental.shard_map import shard_map
    except ImportError:
        from jax.shard_map import shard_map
    import concourse.mybir as mb
    from concourse import bass2jax

    bass2jax.install_neuronx_cc_hook()

    part_name = nc.partition_id_tensor.name if nc.partition_id_tensor else None
    in_names, out_names, out_avals = [], [], []
    for alloc in nc.m.functions[0].allocations:
        if not isinstance(alloc, mb.MemoryLocationSet):
            continue
        name = alloc.memorylocations[0].name
        if alloc.kind == "ExternalInput":
            if name != part_name:
                in_names.append(name)
        elif alloc.kind == "ExternalOutput":
            out_names.append(name)
            out_avals.append(jax.core.ShapedArray(
                tuple(alloc.tensor_shape), mb.dt.np(alloc.dtype)))
    n_params = len(in_names)
    all_names = in_names + out_names
    if part_name is not None:
        all_names = all_names + [part_name]

    def _body(*args):
        operands = list(args)
        if part_name is not None:
            operands.append(bass2jax.partition_id_tensor())
        return tuple(bass2jax._bass_exec_p.bind(
            *operands, out_avals=tuple(out_avals), in_names=tuple(all_names),
            out_names=tuple(out_names), lowering_input_output_aliases=(),
            sim_require_finite=True, sim_require_nnan=True, nc=nc))

    devices = jax.devices()[:B]
    mesh = Mesh(np.asarray(devices), ("core",))
    n_outs = len(out_names)
    sharded = jax.jit(
        shard_map(_body, mesh=mesh,
                  in_specs=(PartitionSpec("core"),) * (n_params + n_outs),
                  out_specs=(PartitionSpec("core"),) * n_outs,
                  check_rep=False),
        donate_argnums=tuple(range(n_params, n_params + n_outs)),
        keep_unused=True)

    def run(in_maps):
        concat_in = [
            np.concatenate([np.asarray(in_maps[c][nm])[None] for c in range(B)],
                           axis=0).reshape(B * in_maps[0][nm].shape[0],
                                           *in_maps[0][nm].shape[1:])
            for nm in in_names
        ]
        concat_zeros = [
            np.zeros((B * av.shape[0], *av.shape[1:]), av.dtype)
            for av in out_avals
        ]
        out_arrs = sharded(*concat_in, *concat_zeros)
        return [
            {nm: np.asarray(out_arrs[i]).reshape(B, *out_avals[i].shape)[c]
             for i, nm in enumerate(out_names)}
            for c in range(B)
        ]

    return run


def kernel(**inputs):
    h_states = np.ascontiguousarray(np.asarray(inputs["h_states"], dtype=np.float32))
    W_heads = np.ascontiguousarray(np.asarray(inputs["W_heads"], dtype=np.float32))
    a_heads = np.ascontiguousarray(np.asarray(inputs["a_heads"], dtype=np.float32))
    W_out = np.ascontiguousarray(np.asarray(inputs["W_out"], dtype=np.float32))
    a_out = np.ascontiguousarray(np.asarray(inputs["a_out"], dtype=np.float32))

    if "nc" not in _NC_CACHE:
        _NC_CACHE["nc"] = build_kernel()
        _NC_CACHE["run"] = _make_runner(_NC_CACHE["nc"])

    xs = h_states.reshape(B, N, D)
    in_maps = [
        {"x": xs[c], "W_heads": W_heads, "a_heads": a_heads,
         "W_out": W_out, "a_out": a_out}
        for c in range(B)
    ]
    results = _NC_CACHE["run"](in_maps)
    return np.concatenate([results[c]["out"] for c in range(B)], axis=0)


if __name__ == "__main__":
    # smoke test (self-contained: random inputs, shape/dtype check only)
    rng = np.random.default_rng(0)
    inputs = {
        "h_states": rng.standard_normal((B * N, D)).astype(np.float32),
        "W_heads": rng.standard_normal((H, D, D)).astype(np.float32) * 0.18,
        "a_heads": rng.standard_normal((H, 2 * D)).astype(np.float32) * 0.18,
        "W_out": rng.standard_normal((H * D, D)).astype(np.float32) * 0.09,
        "a_out": rng.standard_normal((2 * D,)).astype(np.float32) * 0.18,
        "seq_start_end": (np.arange(B, dtype=np.int32)[:, None] * N
                          + np.array([0, N], dtype=np.int32)[None, :]),
    }
    got = kernel(**inputs)
    print("kernel output", got.shape, got.dtype)


# revision 53
# speedup vs baseline: 1.1216x; 1.0616x over previous
"""Bass/Tile Trainium2 kernel for a 2-layer dense multi-head GAT over a batch
of B=8 independent subgraphs (2048 nodes each, equal contiguous segments).

Sharding: one subgraph per NeuronCore (8 cores), parameters replicated.

Algorithm (per core / subgraph, per attention layer):
  scores are rank-1:  e_ij = leaky_relu(s1_i + s2_j),  s1 = h@a1, s2 = h@a2.
  exp(leaky_relu(t)) is separable through the sign mask M_ij = [s1_i+s2_j>=0]:
      p_ij = M_ij e^{s1_i} e^{s2_j} + (1-M_ij) e^{a s1_i} e^{a s2_j}
  so softmax(e) @ h needs NO N^2 exp work:
      num_i = g_i (M @ u)_i + (vtot - (M @ v))_i          (e^{a s1} cancels in
      u_j = e^{s2_j} [h_j|1],  v_j = e^{a s2_j} [h_j|1],   the Z ratio; g =
      out_i = num_i[:64] / num_i[64]                       e^{(1-a) s1})
  The N^2 work is the 0/1 mask build (exact in bf16; DVE is_ge in its 2x
  bf16 mode, with the tail jc columns on GPSIMD) plus mask matmuls with a
  single bf16 [u|-v] stream.  Each head's projections/exponentials are
  emitted inside the previous head's attention quarters so every engine
  stays busy across head boundaries; the layer-2 transposes/projections are
  likewise pipelined into head 3's quarters, and the log_softmax tail is
  per-quarter so the output DMA overlaps the last attention.
"""

from contextlib import ExitStack

import numpy as np

import concourse.bass as bass
import concourse.tile as tile
from concourse import bacc, mybir
from concourse.masks import make_identity

FP = mybir.dt.float32
BF = mybir.dt.bfloat16
AF = mybir.ActivationFunctionType
OP = mybir.AluOpType

B = 8
N = 2048
D = 64
H = 4
ALPHA = 0.2
P = 128
NCH = N // P  # 16 chunks of 128 nodes
DEXT = D + 1  # h plus ones column

# mask ownership: DVE makes every mask tile (GPSIMD is 4.2x slower per mask
# but only ~1.5x slower on elementwise work, so GPSIMD gets uv/onorm/elu and
# DVE gets all masks -- the total-work-minimizing split).
GP_MAIN = frozenset()
GP_SHORT = frozenset()


def _emit_prep_exps(nc, prep, st, cgs):
    """es2/nes2 exponentials for s12 column groups cgs (4 cols each)."""
    for cg in cgs:
        gs = slice(cg * 4, (cg + 1) * 4)
        nc.scalar.activation(st["es2"][:, gs], st["s12"][:, gs, 1], AF.Exp)
        nc.scalar.activation(st["nes2"][:, gs], st["s12"][:, gs, 1], AF.Exp,
                             scale=ALPHA)


def _emit_uv(nc, st, chunks):
    """u | -v bf16 stream for node chunks (the -1 rides the 2nd ALU op)."""
    uv, hext = st["uv"], st["hext"]
    for c in chunks:
        nc.gpsimd.tensor_scalar(uv[:, c, 0:DEXT], hext[:, c, :],
                                st["es2"][:, c:c + 1], None, OP.mult)
        nc.gpsimd.tensor_scalar(uv[:, c, DEXT:], hext[:, c, :],
                                st["nes2"][:, c:c + 1], -1.0, OP.mult, OP.mult)


def _emit_vrows(nc, prep, psaux, scratch, st, tag):
    """vtot = sum_j v_j via PE; bf16 hi+res seed rows [1, 130] (two K=1 seed
    matmuls: ~108ns extra PE per quarter but no DMA-hop latency on the
    layer-critical seed chain)."""
    vt_ps = psaux.tile([1, DEXT], FP, tag="aux", name=f"vt_{tag}")
    for c in range(NCH):
        nc.tensor.matmul(vt_ps, scratch["ones_col_bf"], st["uv"][:, c, DEXT:],
                         start=(c == 0), stop=(c == NCH - 1))
    vrow = prep.tile([1, 2 * DEXT], BF, tag="vrow", name=f"vrow_{tag}")
    nc.vector.memset(vrow[:, 0:DEXT], 0.0)
    nc.vector.tensor_scalar(vrow[:, DEXT:], vt_ps, -1.0, None, OP.mult)
    vres = prep.tile([1, 2 * DEXT], BF, tag="vres", name=f"vres_{tag}")
    nc.vector.memset(vres[:, 0:DEXT], 0.0)
    nc.vector.scalar_tensor_tensor(vres[:, DEXT:], vt_ps, -1.0,
                                   vrow[:, DEXT:], OP.mult, OP.subtract)
    st["vrow"], st["vres"] = vrow, vres


def _emit_gp_masks(nc, mask_gp, st, tag):
    """GPSIMD-owned [P,1024] mask units, emitted as early as possible."""
    tiles = st.setdefault("pre_tiles", {})
    for half, jc in sorted(st["gp_set"]):
        mt = mask_gp.tile([P, 1024], BF, tag="mtg",
                          name=f"mtg{tag}_{half}_{jc}")
        nc.gpsimd.tensor_scalar(
            mt, st["s1b"][:, half * 1024:(half + 1) * 1024],
            st["s12"][:, jc, 1:2], 0.0, OP.add, OP.is_ge)
        tiles[(half, jc)] = mt


def _emit_dve_masks(nc, mask_dve, st, units, tag):
    """Pre-emit DVE mask units (fills DVE idle at layer boundaries)."""
    tiles = st.setdefault("pre_tiles", {})
    for half, jc in units:
        if (half, jc) in tiles or (half, jc) in st["gp_set"]:
            continue
        mt = mask_dve.tile([P, 1024], BF, tag="mtd",
                           name=f"mtd{tag}_{half}_{jc}")
        nc.vector.tensor_scalar(mt,
                                st["s1b"][:, half * 1024:(half + 1) * 1024],
                                st["s12"][:, jc, 1:2], 0.0, OP.add, OP.is_ge)
        tiles[(half, jc)] = mt


def _attention(nc, pools, scratch, st, out_cb, tag, hook=None,
               hook_first=False):
    """Masked-matmul attention core; st holds the layer's prepped tensors."""
    const, prep, mask_dve, mask_gp, wide, small, psA, psaux = pools
    s12, uv = st["s12"], st["uv"]
    g = st["g"]
    ones_row_bf = scratch["ones_row_bf"]

    nsum_w = wide.tile([P, NCH, DEXT], FP, tag="nsum", name=f"nsum_{tag}")
    onorm = wide.tile([P, NCH, D], FP, tag="onorm", name=f"onorm_{tag}")
    LOOKAHEAD = 7
    steps = [(q, jc) for q in range(4) for jc in range(NCH)]
    tiles = dict(st.get("pre_tiles", ()))  # (half, jc) -> [P,1024] tile

    def emit_step(step_idx):
        if step_idx >= len(steps):
            return
        q, jc = steps[step_idx]
        half = q // 2
        if (half, jc) in tiles:
            return
        mt = mask_dve.tile([P, 1024], BF, tag="mtd",
                           name=f"mtd{tag}_{half}_{jc}")
        nc.vector.tensor_scalar(mt, st["s1b"][:, half * 1024:(half + 1) * 1024],
                                s12[:, jc, 1:2], 0.0, OP.add, OP.is_ge)
        tiles[(half, jc)] = mt

    for i in range(LOOKAHEAD):
        emit_step(i)

    for q in range(4):  # quarters of the i (destination-node) axis
        A = [psA.tile([P, 2 * DEXT], FP, tag="A", name=f"A{tag}_{q}_{il}")
             for il in range(4)]
        half, off = q // 2, (q % 2) * 512
        for jc in range(NCH):
            mt = tiles[(half, jc)]
            emit_step(q * NCH + jc + LOOKAHEAD)
            for il in range(4):
                sl = mt[:, off + il * P:off + (il + 1) * P]
                nc.tensor.matmul(A[il], sl, uv[:, jc, :],
                                 start=(jc == 0), stop=False)
        # seed vtot last (PSUM accumulation is order-insensitive)
        for il in range(4):
            nc.tensor.matmul(A[il], ones_row_bf[0:1, :], st["vrow"],
                             start=False, stop=False)
            nc.tensor.matmul(A[il], ones_row_bf[0:1, :], st["vres"],
                             start=False, stop=True)
        qs = slice(q * 4, (q + 1) * 4)
        for il in range(4):
            ic = q * 4 + il
            # nsum = g * (M@u) + (vtot - M@v): ACT evacuates the u-half with
            # the g-scale fused (Copy+scale), DVE adds the PSUM w-half.
            nU = small.tile([P, DEXT], FP, tag="nU", name=f"nU{tag}_{ic}")
            nc.scalar.activation(nU, A[il][:, 0:DEXT], AF.Copy,
                                 scale=g[:, ic:ic + 1])
            nc.vector.tensor_tensor(nsum_w[:, ic, :], nU, A[il][:, DEXT:],
                                    OP.add)
        rz = small.tile([P, 4], FP, tag="rz", name=f"rz{tag}_{q}")
        nc.vector.reciprocal(rz, nsum_w[:, qs, D])
        for k in range(4):
            ic = q * 4 + k
            nc.gpsimd.tensor_scalar(onorm[:, ic, :], nsum_w[:, ic, 0:D],
                                    rz[:, k:k + 1], None, OP.mult)
        if hook is not None and hook_first:
            hook(q)
        out_cb(onorm, q)
        if hook is not None and not hook_first:
            hook(q)


def _elu_q(nc, wide, onorm, q, dst_writer, tag):
    """elu over quarter q of onorm [P, NCH, D]; writes via dst_writer(src)."""
    src = onorm[:, q * 4:(q + 1) * 4, :]
    m = wide.tile([P, 4, D], FP, tag="elu_m", name=f"elu_m{tag}{q}")
    nc.gpsimd.tensor_scalar(m, src, 0.0, None, OP.min)
    e = wide.tile([P, 4, D], FP, tag="elu_e", name=f"elu_e{tag}{q}")
    nc.scalar.activation(e, m, AF.Exp)
    r = wide.tile([P, 4, D], FP, tag="elu_r", name=f"elu_r{tag}{q}")
    nc.gpsimd.tensor_scalar(r, src, 0.0, -1.0, OP.max, OP.add)
    dst_writer(r, e)


def _patch_act_tables():
    """Reorder the activation-table list so the combined exp+ln set is
    preferred: the load-insertion pass is first-fit per function, and with
    the default order it thrashes between the exp-only and ln-only sets.
    Every activation this kernel uses (Exp, Ln, Copy, Identity) lives in
    natural_log_exp_and_others, so one table serves the whole program."""
    import functools
    from concourse import bacc as bacc_mod

    if getattr(bacc_mod, "_gat_tables_patched", False):
        return
    orig = bacc_mod.get_activation_tables

    @functools.cache
    def reordered(arch):
        tabs = orig(arch)
        key = "natural_log_exp_and_others"
        if key not in tabs:
            return tabs
        out = {key: tabs[key]}
        out.update((k, v) for k, v in tabs.items() if k != key)
        return out

    bacc_mod.get_activation_tables = reordered
    bacc_mod._gat_tables_patched = True


def build_kernel():
    nc = bacc.Bacc("TRN2", target_bir_lowering=False, debug=False,
                   num_devices=B)

    x = nc.dram_tensor("x", [N, D], FP, kind="ExternalInput")
    W_heads = nc.dram_tensor("W_heads", [H, D, D], FP, kind="ExternalInput")
    a_heads = nc.dram_tensor("a_heads", [H, 2 * D], FP, kind="ExternalInput")
    W_out = nc.dram_tensor("W_out", [H * D, D], FP, kind="ExternalInput")
    a_out = nc.dram_tensor("a_out", [2 * D], FP, kind="ExternalInput")
    out = nc.dram_tensor("out", [N, D], FP, kind="ExternalOutput")

    with tile.TileContext(nc) as tc, ExitStack() as ctx:
        const = ctx.enter_context(tc.tile_pool(name="const", bufs=1))
        prep = ctx.enter_context(tc.tile_pool(name="prep", bufs=3))
        mask_dve = ctx.enter_context(tc.tile_pool(name="mask_dve", bufs=26))
        mask_gp = ctx.enter_context(tc.tile_pool(name="mask_gp", bufs=2))
        wide = ctx.enter_context(tc.tile_pool(name="wide", bufs=2))
        small = ctx.enter_context(tc.tile_pool(name="small", bufs=6))
        psA = ctx.enter_context(tc.tile_pool(name="psA", bufs=6, space="PSUM"))
        psaux = ctx.enter_context(tc.tile_pool(name="psaux", bufs=2,
                                               space="PSUM"))
        pools = (const, prep, mask_dve, mask_gp, wide, small, psA, psaux)

        ident = const.tile([P, P], FP)
        make_identity(nc, ident)
        ones128 = const.tile([P, P], FP)
        nc.vector.memset(ones128, 1.0)
        ones_col_bf = const.tile([P, 1], BF)
        nc.vector.memset(ones_col_bf, 1.0)
        ones_row_bf = const.tile([2, P], BF)
        nc.vector.memset(ones_row_bf, 1.0)
        scratch = {"ones128": ones128, "ones_col_bf": ones_col_bf,
                   "ones_row_bf": ones_row_bf}

        # ---- load inputs: Wh/a first (they head PE's program order), then x
        # in 4 pieces so transposes start early. WhT/WoT come from cheap PE
        # transposes -- the DMA-side "h k d -> d h k" rearranges cost ~7us in
        # 4-byte descriptors. ----
        Wh = const.tile([64, H, D], FP)
        nc.sync.dma_start(out=Wh, in_=W_heads.rearrange("h k d -> k h d"))
        a_sb = const.tile([64, H, 2], FP)
        nc.sync.dma_start(out=a_sb, in_=a_heads.rearrange("h (t k) -> k h t", t=2))
        x_sb = const.tile([P, NCH, D], FP)
        x_r = x.rearrange("(c p) d -> p c d", p=P)
        for r4 in range(4):
            nc.sync.dma_start(out=x_sb[:, r4 * 4:(r4 + 1) * 4, :],
                              in_=x_r[:, r4 * 4:(r4 + 1) * 4, :])
        Wo = const.tile([P, 2, D], FP)
        nc.sync.dma_start(out=Wo, in_=W_out.rearrange("(c k) d -> k c d", k=P))
        ao = const.tile([64, 2], FP)
        nc.sync.dma_start(out=ao, in_=a_out.rearrange("(t k) -> k t", t=2))

        # bf16 shadows of the moving matmul operands (4x cheaper PE rows)
        Wh_bf = const.tile([64, H, D], BF)
        nc.vector.tensor_copy(Wh_bf, Wh)
        Wo_bf = const.tile([P, 2, D], BF)
        nc.vector.tensor_copy(Wo_bf, Wo)

        # WhT via PE transposes, then wa = W_h @ [a1|a2] (re-association:
        # s = x @ wa); fills the startup bubble before x lands
        WhT = const.tile([64, H, D], FP)
        wa_all = const.tile([64, H, 2], FP)
        for h in range(H):
            tw = psaux.tile([64, D], FP, tag="aux", name=f"tw{h}")
            nc.tensor.transpose(tw, Wh[:, h, :], ident[0:64, 0:64])
            nc.scalar.copy(WhT[:, h, :], tw)
        for h in range(H):
            wap = psaux.tile([64, 2], FP, tag="aux", name=f"wap{h}")
            nc.tensor.matmul(wap, WhT[:, h, :], a_sb[:, h, :], start=True,
                             stop=True)
            nc.scalar.copy(wa_all[:, h, :], wap)

        xT = const.tile([64, N], FP)
        xT_bf = const.tile([64, N], BF)

        def new_state(tag, gp_set):
            st = {"gp_set": gp_set}
            st["s12"] = prep.tile([P, NCH, 2], FP, tag="s12",
                                  name=f"s12_{tag}")
            st["s1b"] = prep.tile([P, N], BF, tag="s1b", name=f"s1b_{tag}")
            st["hext"] = prep.tile([P, NCH, DEXT], BF, tag="hext",
                                   name=f"hext_{tag}")
            nc.vector.memset(st["hext"][:, :, D], 1.0)
            st["uv"] = prep.tile([P, NCH, 2 * DEXT], BF, tag="uv",
                                 name=f"uv_{tag}")
            st["es2"] = prep.tile([P, NCH], FP, tag="es2", name=f"es2_{tag}")
            st["nes2"] = prep.tile([P, NCH], FP, tag="nes2",
                                   name=f"nes2_{tag}")
            st["g"] = prep.tile([P, NCH], FP, tag="g", name=f"g_{tag}")
            return st

        def l1_partA(st, h, cgs=range(4)):
            # s12 columns (batched copies, 4 chunks per PSUM tile) + exps
            wa = wa_all[:, h, :]
            for cg in cgs:
                sp = psaux.tile([P, 8], FP, tag="aux", name=f"sp{h}_{cg}")
                for k in range(4):
                    c = cg * 4 + k
                    nc.tensor.matmul(sp[:, 2 * k:2 * k + 2],
                                     xT[:, c * P:(c + 1) * P], wa,
                                     start=True, stop=True)
                nc.scalar.copy(st["s12"][:, cg * 4:(cg + 1) * 4, :], sp)
                _emit_prep_exps(nc, prep, st, [cg])

        def l1_partB(st, h, rs=range(4), halves=range(2), premask=False):
            # s1b (bf16 mask input): s1 row replicated via (ones*wa1) x xT
            wa = wa_all[:, h, :]
            wa1b = prep.tile([64, P], BF, tag="wa1b", name=f"wa1b_{h}")
            nc.vector.tensor_scalar(wa1b, ones128[0:64, :], wa[:, 0:1], None,
                                    OP.mult)
            for r in rs:
                ps = psaux.tile([P, 512], FP, tag="aux", name=f"s1p{h}_{r}")
                nc.tensor.matmul(ps, wa1b, xT_bf[:, r * 512:(r + 1) * 512],
                                 start=True, stop=True)
                nc.scalar.copy(st["s1b"][:, r * 512:(r + 1) * 512], ps)
            # h natural (+ones col), bf16, batched 8 chunks per PSUM bank,
            # with the uv stream built right behind each half
            for half in halves:
                hp = psaux.tile([P, 8, D], FP, tag="aux", name=f"hp{h}_{half}")
                for k in range(8):
                    c = half * 8 + k
                    nc.tensor.matmul(hp[:, k, :], xT_bf[:, c * P:(c + 1) * P],
                                     Wh_bf[:, h, :], start=(k == 0),
                                     stop=(k == 7))
                nc.scalar.copy(st["hext"][:, half * 8:(half + 1) * 8, 0:D], hp)
                _emit_uv(nc, st, range(half * 8, half * 8 + 8))
            if premask:
                nc.scalar.activation(st["g"], st["s12"][:, :, 0], AF.Exp,
                                     scale=1.0 - ALPHA)
                _emit_dve_masks(nc, mask_dve, st,
                                [(0, jc) for jc in range(8)], f"h{h}")

        def l1_partC(st, h):
            _emit_vrows(nc, prep, psaux, scratch, st, f"h{h}")
            _emit_dve_masks(nc, mask_dve, st,
                            [(0, jc) for jc in range(8, 12)], f"h{h}")

        # ---- startup: interleave x transposes with head-0 prep ----
        st0 = new_state("h0", GP_SHORT)
        for r4 in range(4):
            for k in range(4):
                c = r4 * 4 + k
                tp = psaux.tile([64, P], FP, tag="aux", name=f"tp{c}")
                nc.tensor.transpose(tp, x_sb[:, c, :], ident)
                if c % 2 == 0:
                    nc.vector.tensor_copy(xT[:, c * P:(c + 1) * P], tp)
                else:
                    nc.scalar.copy(xT[:, c * P:(c + 1) * P], tp)
            nc.vector.tensor_copy(xT_bf[:, r4 * 512:(r4 + 1) * 512],
                                  xT[:, r4 * 512:(r4 + 1) * 512])
            l1_partA(st0, 0, cgs=[r4])
            l1_partB(st0, 0, rs=[r4], halves=[])
        l1_partB(st0, 0, rs=[], halves=range(2), premask=True)
        l1_partC(st0, 0)
        # WoT via PE transposes (needed first at head-3's l2 hooks)
        WoT = const.tile([64, 2, P], FP)
        for kc in range(2):
            to = psaux.tile([64, P], FP, tag="aux", name=f"to{kc}")
            nc.tensor.transpose(to, Wo[:, kc, :], ident)
            nc.scalar.copy(WoT[:, kc, :], to)

        # ---- layer 1: four heads -> xc01/xc23 (bf16: halves the transpose
        # row cost and the evac copies run in DVE 2x mode) ----
        xc01 = const.tile([P, NCH, 2, D], BF)
        xc23 = const.tile([P, NCH, 2, D], BF)
        xcT_bf = const.tile([P, 2, N], BF)
        ident_bf = const.tile([P, P], BF)
        nc.vector.tensor_copy(ident_bf, ident)

        # layer-2 state (filled by head-3 hooks)
        st2 = None
        states = [st0]

        def l2_piece(cg):
            """Layer-2 prep for xc chunk group cg (0..3): transpose + the
            projections that depend on those chunks. Invoked one quarter
            AFTER head 3 writes the group, so the transposes never stall PE
            on the epilogue chain."""
            st = st2
            for c in range(cg * 4, cg * 4 + 4):
                for kc, xc in ((0, xc01), (1, xc23)):
                    tp = psaux.tile([P, P], BF, tag="aux", name=f"tc{c}_{kc}")
                    nc.tensor.transpose(tp, xc[:, c, :, :], ident_bf)
                    if (c + kc) % 2 == 0:
                        nc.vector.tensor_copy(
                            xcT_bf[:, kc, c * P:(c + 1) * P], tp)
                    else:
                        nc.scalar.copy(xcT_bf[:, kc, c * P:(c + 1) * P], tp)
            if cg == 0:
                # wa2 = W_out @ [a1|a2] (bf16 shadow for the bf16 stationary)
                wa2 = prep.tile([P, 2, 2], FP, tag="wa2")
                for kc in range(2):
                    wap = psaux.tile([P, 2], FP, tag="aux", name=f"wap2_{kc}")
                    nc.tensor.matmul(wap, WoT[:, kc, :], ao, start=True,
                                     stop=True)
                    nc.scalar.copy(wa2[:, kc, :], wap)
                wa2b = prep.tile([P, 2, 2], BF, tag="wa2b")
                nc.vector.tensor_copy(wa2b, wa2)
                st["wa2b"] = wa2b
                wa1b2 = prep.tile([P, 2, P], BF, tag="wa1b2")
                for kc in range(2):
                    nc.vector.tensor_scalar(wa1b2[:, kc, :], ones128,
                                            wa2[:, kc, 0:1], None, OP.mult)
                st["wa1b2"] = wa1b2
            sp = psaux.tile([P, 8], FP, tag="aux", name=f"sp2_{cg}")
            for k in range(4):
                c = cg * 4 + k
                for kc in range(2):
                    nc.tensor.matmul(sp[:, 2 * k:2 * k + 2],
                                     xcT_bf[:, kc, c * P:(c + 1) * P],
                                     st["wa2b"][:, kc, :],
                                     start=(kc == 0), stop=(kc == 1))
            nc.scalar.copy(st["s12"][:, cg * 4:(cg + 1) * 4, :], sp)
            _emit_prep_exps(nc, prep, st, [cg])
            ps = psaux.tile([P, 512], FP, tag="aux", name=f"s1p2_{cg}")
            for kc in range(2):
                nc.tensor.matmul(ps, st["wa1b2"][:, kc, :],
                                 xcT_bf[:, kc, cg * 512:(cg + 1) * 512],
                                 start=(kc == 0), stop=(kc == 1))
            nc.scalar.copy(st["s1b"][:, cg * 512:(cg + 1) * 512], ps)
            # h2ext halves once their 8 chunks are transposed
            if cg in (1, 3):
                half = cg // 2
                hp = psaux.tile([P, 8, D], FP, tag="aux", name=f"hp2_{half}")
                for k in range(8):
                    c = half * 8 + k
                    for kc in range(2):
                        nc.tensor.matmul(hp[:, k, :],
                                         xcT_bf[:, kc, c * P:(c + 1) * P],
                                         Wo_bf[:, kc, :],
                                         start=(k == 0 and kc == 0),
                                         stop=(k == 7 and kc == 1))
                nc.scalar.copy(st["hext"][:, half * 8:(half + 1) * 8, 0:D],
                               hp)
                _emit_uv(nc, st, range(half * 8, half * 8 + 8))
            if cg == 1:
                _emit_dve_masks(nc, mask_dve, st,
                                [(0, jc) for jc in range(6)], "l2")
            if cg == 3:
                nc.scalar.activation(st["g"], st["s12"][:, :, 0], AF.Exp,
                                     scale=1.0 - ALPHA)
                _emit_dve_masks(nc, mask_dve, st,
                                [(0, jc) for jc in range(6, 12)], "l2")
                _emit_vrows(nc, prep, psaux, scratch, st, "l2")

        for h in range(H):
            st = states[h]

            def l1_out(onorm, q, h=h):
                xc = xc01 if h < 2 else xc23

                def write(r, e):
                    nc.gpsimd.tensor_tensor(
                        xc[:, q * 4:(q + 1) * 4, h % 2, :], r, e, OP.add)

                _elu_q(nc, wide, onorm, q, write, f"h{h}")

            if h < H - 1:
                nst = new_state(f"h{h + 1}", GP_MAIN)
                states.append(nst)

                def hook(q, h=h, nst=nst):
                    if q == 1:
                        l1_partA(nst, h + 1)
                    elif q == 2:
                        l1_partB(nst, h + 1, premask=True)
                    elif q == 3:
                        l1_partC(nst, h + 1)
            else:
                st2 = new_state("l2", GP_SHORT)

                def hook(q):
                    if q >= 1:
                        l2_piece(q - 1)

            _attention(nc, pools, scratch, st, l1_out, f"h{h}", hook,
                       hook_first=True)
        l2_piece(3)

        # ---- layer 2 attention + elu + per-quarter log_softmax -> out ----
        out_w = const.tile([P, NCH, D], FP)
        out_r = out.rearrange("(c p) d -> p c d", p=P)
        o2_all = const.tile([P, NCH, D], FP)
        esum_all = const.tile([P, NCH], FP)
        lse = const.tile([P, NCH], FP)

        def l2_out(onorm, q):
            # elu + raw exp-sum (elu output is <= ~20, so exp is fp32-safe
            # without max subtraction); Ln + final subtract deferred so the
            # Exp/Ln ACT tables swap once, not per quarter
            qs = slice(q * 4, (q + 1) * 4)
            o2 = o2_all[:, qs, :]

            def write(r, e):
                nc.gpsimd.tensor_tensor(o2, r, e, OP.add)

            _elu_q(nc, wide, onorm, q, write, "l2")
            escr = wide.tile([P, 4, D], FP, tag="escr", name=f"escr{q}")
            for k in range(4):
                ic = q * 4 + k
                nc.scalar.activation(escr[:, k, :], o2[:, k, :], AF.Exp,
                                     accum_out=esum_all[:, ic:ic + 1])

        _attention(nc, pools, scratch, st2, l2_out, "l2")

        nc.scalar.activation(lse, esum_all, AF.Ln)
        for q in range(4):
            qs = slice(q * 4, (q + 1) * 4)
            lse_b = bass.AP(tensor=lse.tensor, offset=lse.offset + q * 4,
                            ap=[lse.ap[0], [lse.ap[1][0], 4], [0, D]])
            nc.vector.tensor_tensor(out_w[:, qs, :], o2_all[:, qs, :], lse_b,
                                    OP.subtract)
            nc.sync.dma_start(out=out_r[:, qs, :], in_=out_w[:, qs, :])

    nc.compile()
    return nc


_NC_CACHE = {}


def _make_runner(nc):
    """Build a cached sharded executable (run_bass_kernel_spmd re-traces
    jax.jit on every call; this jits once and reuses)."""
    import jax
    from jax.sharding import Mesh, PartitionSpec
    try:
        from jax.experimental.shard_map import shard_map
    except ImportError:
        from jax.shard_map import shard_map
    import concourse.mybir as mb
    from concourse import bass2jax

    bass2jax.install_neuronx_cc_hook()

    part_name = nc.partition_id_tensor.name if nc.partition_id_tensor else None
    in_names, out_names, out_avals = [], [], []
    for alloc in nc.m.functions[0].allocations:
        if not isinstance(alloc, mb.MemoryLocationSet):
            continue
        name = alloc.memorylocations[0].name
        if alloc.kind == "ExternalInput":
            if name != part_name:
                in_names.append(name)
        elif alloc.kind == "ExternalOutput":
            out_names.append(name)
            out_avals.append(jax.core.ShapedArray(
                tuple(alloc.tensor_shape), mb.dt.np(alloc.dtype)))
    n_params = len(in_names)
    all_names = in_names + out_names
    if part_name is not None:
        all_names = all_names + [part_name]

    def _body(*args):
        operands = list(args)
        if part_name is not None:
            operands.append(bass2jax.partition_id_tensor())
        return tuple(bass2jax._bass_exec_p.bind(
            *operands, out_avals=tuple(out_avals), in_names=tuple(all_names),
            out_names=tuple(out_names), lowering_input_output_aliases=(),
            sim_require_finite=True, sim_require_nnan=True, nc=nc))

    devices = jax.devices()[:B]
    mesh = Mesh(np.asarray(devices), ("core",))
    n_outs = len(out_names)
    sharded = jax.jit(
        shard_map(_body, mesh=mesh,
                  in_specs=(PartitionSpec("core"),) * (n_params + n_outs),
                  out_specs=(PartitionSpec("core"),) * n_outs,
                  check_rep=False),
        donate_argnums=tuple(range(n_params, n_params + n_outs)),
        keep_unused=True)

    def run(in_maps):
        concat_in = [
            np.concatenate([np.asarray(in_maps[c][nm])[None] for c in range(B)],
                           axis=0).reshape(B * in_maps[0][nm].shape[0],
                                           *in_maps[0][nm].shape[1:])
            for nm in in_names
        ]
        concat_zeros = [
            np.zeros((B * av.shape[0], *av.shape[1:]), av.dtype)
            for av in out_avals
        ]
        out_arrs = sharded(*concat_in, *concat_zeros)
        return [
            {nm: np.asarray(out_arrs[i]).reshape(B, *out_avals[i].shape)[c]
             for i, nm in enumerate(out_names)}
            for c in range(B)
        ]

    return run


def kernel(**inputs):
    h_states = np.ascontiguousarray(np.asarray(inputs["h_states"], dtype=np.float32))
    W_heads = np.ascontiguousarray(np.asarray(inputs["W_heads"], dtype=np.float32))
    a_heads = np.ascontiguousarray(np.asarray(inputs["a_heads"], dtype=np.float32))
    W_out = np.ascontiguousarray(np.asarray(inputs["W_out"], dtype=np.float32))
    a_out = np.ascontiguousarray(np.asarray(inputs["a_out"], dtype=np.float32))

    if "nc" not in _NC_CACHE:
        _NC_CACHE["nc"] = build_kernel()
        _NC_CACHE["run"] = _make_runner(_NC_CACHE["nc"])

    xs = h_states.reshape(B, N, D)
    in_maps = [
        {"x": xs[c], "W_heads": W_heads, "a_heads": a_heads,
         "W_out": W_out, "a_out": a_out}
        for c in range(B)
    ]
    results = _NC_CACHE["run"](in_maps)
    return np.concatenate([results[c]["out"] for c in range(B)], axis=0)


if __name__ == "__main__":
    # smoke test (self-contained: random inputs, shape/dtype check only)
    rng = np.random.default_rng(0)
    inputs = {
        "h_states": rng.standard_normal((B * N, D)).astype(np.float32),
        "W_heads": rng.standard_normal((H, D, D)).astype(np.float32) * 0.18,
        "a_heads": rng.standard_normal((H, 2 * D)).astype(np.float32) * 0.18,
        "W_out": rng.standard_normal((H * D, D)).astype(np.float32) * 0.09,
        "a_out": rng.standard_normal((2 * D,)).astype(np.float32) * 0.18,
        "seq_start_end": (np.arange(B, dtype=np.int32)[:, None] * N
                          + np.array([0, N], dtype=np.int32)[None, :]),
    }
    got = kernel(**inputs)
    print("kernel output", got.shape, got.dtype)
